# revision 7
# baseline (speedup 1.0000x reference)
"""Gated linear attention on 8 TRN2 NeuronCores.

Sharding: data-parallel over tokens. Core c handles tokens
[c*2048, (c+1)*2048) of the flattened (B*N, C) = (16384, 1024) sequence,
i.e. batch b = c//2, sequence half = c%2. The linear-attention kv state
(and k-sum) needs a reduction over each batch's full sequence, so cores
{2b, 2b+1} all-reduce a small (128, 520) fp32 buffer (kv state + k-sum
for 16 heads) and everything else is local.

Device layouts (per core):
  xt   [C, T]    bf16  x^T shard (host pre-transposes + casts)
  wg   [C, C]    bf16  Wg   (lhsT col-blocks for gate-proj, out [g, tok])
  wkv  [C, 2C]   bf16  Wqkv[:, C:3C]  (rhs for k/v-proj, out [tok, feat])
  wq   [C, C]    bf16  Wqkv[:, :C]    (lhsT for q-proj, out [d, tok])
  wp   [C, C]    bf16  Wp             (lhsT for out-proj, out [o, tok])
  y    [C, T]    bf16  output^T (host transposes back + casts fp32)

q and gates are computed feature-major ([feat, tok]); k and v token-major
([tok, feat]) so the kv einsum can contract over tokens on the partition
axis. Gates are transposed on the DMA xbar (bf16 128x128 tiles) for the
k side. elu(z)+1 is computed as min(exp(z),1) + max(z,0), with exp and
relu on ACT and the mul/combine on DVE so neither engine gates the PE.

v2 scheduling (vs the first working version):
 - gates use k-innermost groups over (m-pair x n-set) PSUM banks so the
   first matmul only needs wg[k=0] + xt[k=0] (768KB) instead of the full
   6MB -> startup stall ~3us instead of ~19us.
 - one 8-buffer PSUM pool; all psum tiles are [128,512]f32 (1 bank).
 - elu combine on GpSimd in the k/v phase (DVE was saturated), on DVE in
   the q phase (GpSimd queue must stay clear for the collective trigger).
 - einsum for 512-token group g is emitted after kvproj chunk 4g+4 so its
   lhsT (DVE/GpSimd products) are ready when the PE reaches it.
 - attention for chunk n+1 is emitted before projection of chunk n, and
   output staging is a ping-pong buffer so the final ACT->DMA chain never
   serializes against DMA completion (this removed a ~23us tail).
"""

import numpy as np
import ml_dtypes

import concourse.bass as bass
import concourse.bacc as bacc
import concourse.tile as tile
import concourse.mybir as mybir
from concourse.bass_utils import run_bass_kernel_spmd

F32 = mybir.dt.float32
BF16 = mybir.dt.bfloat16
AF = mybir.ActivationFunctionType
ALU = mybir.AluOpType

B, N, C = 4, 4096, 1024
H, D = 16, 64
NCORES = 8
T = B * N // NCORES          # 2048 tokens per core
KC = C // 128                # 8 contraction chunks
TB = 512                     # token tile (free dim)
NT = T // TB                 # 4 token tiles
NS = T // 128                # 16 token subchunks (partition-dim tiles)
C2 = 2 * C

REPLICA_GROUPS = [[0, 1], [2, 3], [4, 5], [6, 7]]


def build_nc():
    nc = bacc.Bacc(
        "TRN2", target_bir_lowering=False, debug=False, num_devices=NCORES
    )
    xt = nc.dram_tensor("xt", [C, T], BF16, kind="ExternalInput")
    wq = nc.dram_tensor("wq", [C, C], BF16, kind="ExternalInput")
    wkv = nc.dram_tensor("wkv", [C, C2], BF16, kind="ExternalInput")
    wgt = nc.dram_tensor("wgt", [C, C], BF16, kind="ExternalInput")
    wp = nc.dram_tensor("wp", [C, C], BF16, kind="ExternalInput")
    bgbp = nc.dram_tensor("bgbp", [128, 2 * KC], F32, kind="ExternalInput")
    e_all = nc.dram_tensor("e_all", [H, C], BF16, kind="ExternalInput")
    y = nc.dram_tensor("y", [C, T], BF16, kind="ExternalOutput")

    with tile.TileContext(nc) as tc:
        build_body(nc, tc, xt, wq, wkv, wgt, wp, bgbp, e_all, y)

    nc.compile()
    return nc


def build_body(nc, tc, xt, wq, wkv, wgt, wp, bgbp, e_all, y):
    from contextlib import ExitStack

    with ExitStack() as st:
        constp = st.enter_context(tc.tile_pool(name="constp", bufs=1))
        wbig = st.enter_context(tc.tile_pool(name="wbig", bufs=1))
        wsmall = st.enter_context(tc.tile_pool(name="wsmall", bufs=2))
        big1 = st.enter_context(tc.tile_pool(name="big1", bufs=1))
        gatesp = st.enter_context(tc.tile_pool(name="gatesp", bufs=1))
        qp = st.enter_context(tc.tile_pool(name="qp", bufs=1))
        workp = st.enter_context(tc.tile_pool(name="workp", bufs=2))
        elup = st.enter_context(tc.tile_pool(name="elup", bufs=2))
        psum = st.enter_context(tc.tile_pool(name="psum", bufs=8, space="PSUM"))
        dramp = st.enter_context(tc.tile_pool(name="dramp", bufs=1, space="DRAM"))

        # ---------------------------------------------- initial loads
        # emission order == sync-queue issue order; gates consume wg/xt
        # chunk pairs k-innermost, so interleave them for tight pacing.
        wg_sb = wsmall.tile([128, KC * C], BF16, name="wg_sb", tag="wsmall")
        xt_sb = big1.tile([128, KC * T], BF16, name="xt_sb", tag="big1")
        wkv_sb = wbig.tile([128, KC * C2], BF16, name="wkv_sb", tag="wbig")
        for k in range(KC):
            nc.sync.dma_start(
                wg_sb[:, k * C : (k + 1) * C], wgt[k * 128 : (k + 1) * 128, :]
            )
            nc.sync.dma_start(
                xt_sb[:, k * T : (k + 1) * T], xt[k * 128 : (k + 1) * 128, :]
            )
        for k in range(KC):
            nc.sync.dma_start(
                wkv_sb[:, k * C2 : (k + 1) * C2], wkv[k * 128 : (k + 1) * 128, :]
            )
        bgbp_sb = constp.tile([128, 2 * KC], F32, name="bgbp_sb")
        nc.sync.dma_start(bgbp_sb[:], bgbp[:])
        e_sb = constp.tile([H, C], BF16, name="e_sb")
        nc.sync.dma_start(e_sb[:], e_all[:])
        # wq goes into the second wsmall slot: free from the start
        wq_sb = wsmall.tile([128, KC * C], BF16, name="wq_sb", tag="wsmall")
        nc.sync.dma_start(
            wq_sb.rearrange("p (k n) -> p k n", k=KC),
            wq.rearrange("(k p) n -> p k n", p=128),
        )

        # ---------------------------------------------- phase 1: gates
        # gates[g, tok] = sigmoid(x @ Wg + bg)^T, feature-major.
        # Groups hold (m, n) PSUM banks across the k-innermost accumulation;
        # G0 spans all 4 token tiles for m-pair 0 (8 banks) to maximize
        # per-k-chunk work while the wg/xt DMA stream is still arriving.
        gates_sb = gatesp.tile([128, KC * T], BF16, name="gates_sb", tag="gates")
        gT_full = qp.tile([128, KC * T], BF16, name="gT_full", tag="gT_full")
        gT4 = gT_full.rearrange("p (m s c) -> p m s c", s=NS, c=128)

        groups = [
            (0, (0, 1, 2, 3)),
            (1, (0, 1)), (2, (0, 1)), (3, (0, 1)),
            (1, (2, 3)), (2, (2, 3)), (3, (2, 3)),
        ]
        for mp, ns in groups:
            ms = (2 * mp, 2 * mp + 1)
            tiles = {}
            for m in ms:
                for n in ns:
                    tiles[(m, n)] = psum.tile(
                        [128, TB], F32, name=f"gps{m}_{n}", tag="mm"
                    )
            for k in range(KC):
                for m in ms:
                    lhsT = wg_sb[:, k * C + m * 128 : k * C + (m + 1) * 128]
                    for n in ns:
                        nc.tensor.matmul(
                            tiles[(m, n)][:],
                            lhsT=lhsT,
                            rhs=xt_sb[:, k * T + n * TB : k * T + (n + 1) * TB],
                            start=(k == 0),
                            stop=(k == KC - 1),
                        )
            for m in ms:
                for n in ns:
                    nc.scalar.activation(
                        gates_sb[:, m * T + n * TB : m * T + (n + 1) * TB],
                        tiles[(m, n)][:],
                        AF.Sigmoid,
                        bias=bgbp_sb[:, m : m + 1],
                        scale=1.0,
                    )
            # transpose the finished half-rows (or full rows for G0) on the
            # DMA xbar: gT[p, m*T + s*128 + c] = gates[m*128 + c, s*128 + p]
            span = len(ns) * TB
            base = ns[0] * TB
            for m in ms:
                nc.sync.dma_start(
                    gT_full[:, m * T + base : m * T + base + span].rearrange(
                        "p (s c) -> p s c", c=128
                    ),
                    gates_sb[:, m * T + base : m * T + base + span],
                    transpose=True,
                )

        # wp into the wsmall slot vacated by wg (waits for last gate matmul)
        wp_sb = wsmall.tile([128, KC * C], BF16, name="wp_sb", tag="wsmall")
        nc.sync.dma_start(
            wp_sb.rearrange("p (k n) -> p k n", k=KC),
            wp.rearrange("(k p) n -> p k n", p=128),
        )

        # ---------------------------------------------- phase 2: k/v + kv state
        # kv_acc block p = cols [130p, 130p+130):
        #   rows 0:64,  cols +0:65   = kv_aug head 2p   (col 64 = k_sum)
        #   rows 64:128, cols +65:130 = kv_aug head 2p+1 (col 129 = k_sum)
        kv_acc = constp.tile([128, KC * 130], F32, name="kv_acc")
        kbfs, vaugs = {}, {}

        def kvproj(s):
            kvps = [
                psum.tile([128, TB], F32, name=f"kvps{n}", tag="mm")
                for n in range(4)
            ]
            for k in range(KC):
                lhsT = xt_sb[:, k * T + s * 128 : k * T + (s + 1) * 128]
                for n in range(4):
                    nc.tensor.matmul(
                        kvps[n][:],
                        lhsT=lhsT,
                        rhs=wkv_sb[:, k * C2 + n * TB : k * C2 + (n + 1) * TB],
                        start=(k == 0),
                        stop=(k == KC - 1),
                    )
            # k = elu(k_raw * g) + 1 = min(exp(kg),1) + max(kg,0)
            k_bf = workp.tile([128, C], BF16, name="k_bf", tag="k_bf", bufs=5)
            for n in range(2):
                kg = elup.tile([128, TB], BF16, name="kg", tag="kg")
                nc.vector.tensor_mul(
                    kg.rearrange("p (m c) -> p m c", c=128),
                    kvps[n].rearrange("p (m c) -> p m c", c=128),
                    gT4[:, 4 * n : 4 * n + 4, s, :],
                )
                relu = elup.tile([128, TB], BF16, name="relu", tag="relu")
                nc.scalar.activation(relu[:], kg[:], AF.Relu)
                ex = elup.tile([128, TB], BF16, name="ex", tag="ex")
                nc.scalar.activation(ex[:], kg[:], AF.Exp)
                nc.vector.scalar_tensor_tensor(
                    k_bf[:, n * TB : (n + 1) * TB],
                    in0=ex[:],
                    scalar=1.0,
                    in1=relu[:],
                    op0=ALU.min,
                    op1=ALU.add,
                )
            # v, augmented with ones column per head (yields k_sum)
            v_aug = workp.tile(
                [128, H * 65], BF16, name="v_aug", tag="v_aug", bufs=5
            )
            v3 = v_aug.rearrange("p (h e) -> p h e", e=65)
            nc.vector.memset(v3[:, :, 64:65], 1.0)
            for n in range(2, 4):
                h0 = (n - 2) * 8
                nc.vector.tensor_copy(
                    v3[:, h0 : h0 + 8, 0:64],
                    kvps[n].rearrange("p (h e) -> p h e", e=64),
                )
            kbfs[s] = k_bf
            vaugs[s] = v_aug

        def einsum(g):
            for p in range(KC):
                eps = psum.tile([128, TB], F32, name="eps", tag="mm")
                for si in range(4):
                    s = g * 4 + si
                    nc.tensor.matmul(
                        eps[:, 0:130],
                        lhsT=kbfs[s][:, 128 * p : 128 * (p + 1)],
                        rhs=vaugs[s][:, 130 * p : 130 * (p + 1)],
                        start=(si == 0),
                        stop=(si == 3),
                    )
                if g == 0:
                    nc.vector.tensor_copy(
                        kv_acc[:, 130 * p : 130 * (p + 1)], eps[:, 0:130]
                    )
                else:
                    nc.vector.tensor_add(
                        kv_acc[:, 130 * p : 130 * (p + 1)],
                        kv_acc[:, 130 * p : 130 * (p + 1)],
                        eps[:, 0:130],
                    )

        # einsum g is emitted after kvproj 4g+4 so the PE never waits on the
        # elu chain of the group's last chunk.
        for s in range(NS):
            kvproj(s)
            if s in (5, 9, 13):
                einsum((s - 5) // 4)
        einsum(3)

        # ---------------------------------------------- kv all-reduce (pairs)
        # compact to [128, 8*65]: head 2p at [0:64, 65p:65p+65],
        # head 2p+1 at [64:128, 65p:65p+65]
        kv_cat = constp.tile([128, KC * 65], F32, name="kv_cat", tag="kv_cat")
        nc.vector.tensor_copy(
            kv_cat[0:64, :].rearrange("p (j e) -> p j e", e=65),
            kv_acc[0:64, :].rearrange("p (j q) -> p j q", q=130)[:, :, 0:65],
        )
        nc.vector.tensor_copy(
            kv_cat[64:128, :].rearrange("p (j e) -> p j e", e=65),
            kv_acc[64:128, :].rearrange("p (j q) -> p j q", q=130)[:, :, 65:130],
        )
        bounce_in = dramp.tile([128, KC * 65], F32, name="bounce_in")
        bounce_out = dramp.tile([128, KC * 65], F32, name="bounce_out")
        nc.sync.dma_start(bounce_in[:], kv_cat[:])
        nc.gpsimd.collective_compute(
            "AllReduce",
            ALU.add,
            replica_groups=REPLICA_GROUPS,
            ins=[bounce_in.opt()],
            outs=[bounce_out.opt()],
        )

        # ---------------------------------------------- phase 3: q (overlaps AR)
        # q feature-major in (m-pair x n-pair) 4-bank groups, k innermost.
        # elu combine stays on DVE here: the gpsimd queue holds the
        # collective trigger and must not be backed up behind q's work.
        q_sb = wbig.tile([128, KC * T], BF16, name="q_sb", tag="wbig")
        for mp in range(4):
            for np_ in range(2):
                ms = (2 * mp, 2 * mp + 1)
                ns = (2 * np_, 2 * np_ + 1)
                tiles = {}
                for m in ms:
                    for n in ns:
                        tiles[(m, n)] = psum.tile(
                            [128, TB], F32, name=f"qps{m}_{n}", tag="mm"
                        )
                for k in range(KC):
                    for m in ms:
                        lhsT = wq_sb[:, k * C + m * 128 : k * C + (m + 1) * 128]
                        for n in ns:
                            nc.tensor.matmul(
                                tiles[(m, n)][:],
                                lhsT=lhsT,
                                rhs=xt_sb[:, k * T + n * TB : k * T + (n + 1) * TB],
                                start=(k == 0),
                                stop=(k == KC - 1),
                            )
                for m in ms:
                    for n in ns:
                        qg = elup.tile([128, TB], BF16, name="qg", tag="kg")
                        nc.vector.tensor_mul(
                            qg[:],
                            tiles[(m, n)][:],
                            gates_sb[:, m * T + n * TB : m * T + (n + 1) * TB],
                        )
                        relu = elup.tile([128, TB], BF16, name="relu2", tag="relu")
                        nc.scalar.activation(relu[:], qg[:], AF.Relu)
                        ex = elup.tile([128, TB], BF16, name="ex2", tag="ex")
                        nc.scalar.activation(ex[:], qg[:], AF.Exp)
                        nc.vector.scalar_tensor_tensor(
                            q_sb[:, m * T + n * TB : m * T + (n + 1) * TB],
                            in0=ex[:],
                            scalar=1.0,
                            in1=relu[:],
                            op0=ALU.min,
                            op1=ALU.add,
                        )

        # ---------------------------------------------- AR result -> kv tiles
        kv_f32 = constp.tile([128, KC * 65], F32, name="kv_f32", tag="kv_cat")
        nc.sync.dma_start(kv_f32[:], bounce_out[:])
        kv_bf = constp.tile([128, KC * 65], BF16, name="kv_bf")
        nc.vector.tensor_copy(kv_bf[:], kv_f32[:])
        # block-diagonal kv tiles for the attention matmul (K=128 per pair)
        bds = []
        for j in range(KC):
            bd = constp.tile([128, 128], BF16, name=f"bd{j}")
            nc.vector.memset(bd[:], 0.0)
            nc.vector.tensor_copy(bd[0:64, 0:64], kv_bf[0:64, 65 * j : 65 * j + 64])
            nc.vector.tensor_copy(
                bd[64:128, 64:128], kv_bf[64:128, 65 * j : 65 * j + 64]
            )
            bds.append(bd)
        # block-diagonal k_sum tiles for the normalizer matmul
        blks = []
        for j in range(KC):
            bj = constp.tile([128, H], BF16, name=f"blk{j}")
            nc.vector.memset(bj[:], 0.0)
            for par in range(2):
                h = 2 * j + par
                nc.vector.tensor_copy(
                    bj[par * 64 : (par + 1) * 64, h : h + 1],
                    kv_bf[par * 64 : (par + 1) * 64, 65 * j + 64 : 65 * j + 65],
                )
            blks.append(bj)

        # ---------------------------------------------- phase 4: attention + proj
        # normalizer reciprocals for all token chunks up front
        rbs = []
        for n in range(NT):
            nps = psum.tile([128, TB], F32, name="nps", tag="mm")
            for j in range(KC):
                nc.tensor.matmul(
                    nps[0:H, :],
                    lhsT=blks[j][:],
                    rhs=q_sb[:, j * T + n * TB : j * T + (n + 1) * TB],
                    start=(j == 0),
                    stop=(j == KC - 1),
                )
            nc.vector.tensor_scalar_add(nps[0:H, :], nps[0:H, :], 1e-8)
            nrec = elup.tile([H, TB], F32, name="nrec", tag="nrec")
            nc.vector.reciprocal_approx_fast(nrec[:], nps[0:H, :])
            rb = constp.tile([H, TB], BF16, name=f"rb{n}")
            nc.vector.tensor_copy(rb[:], nrec[:])
            rbs.append(rb)

        # ping-pong buffers: attention output (feature-major) and staged y
        attn_pp = big1.tile([128, 2 * KC * TB], BF16, name="attn_pp", tag="big1")
        o_pp = gatesp.tile([128, 2 * KC * TB], BF16, name="o_pp", tag="gates")

        def attn_emit(n):
            # attn[e, tok] = (q @ kv) * bcast(recip)   (feature-major)
            half = (n % 2) * KC * TB
            for j in range(KC):
                pps = psum.tile([128, TB], F32, name="pps", tag="mm")
                nc.tensor.matmul(
                    pps[:],
                    lhsT=bds[j][:],
                    rhs=q_sb[:, j * T + n * TB : j * T + (n + 1) * TB],
                    start=True,
                    stop=True,
                )
                bps = psum.tile([128, TB], F32, name="bps", tag="mm")
                nc.tensor.matmul(
                    bps[:],
                    lhsT=e_sb[:, j * 128 : (j + 1) * 128],
                    rhs=rbs[n][:],
                    start=True,
                    stop=True,
                )
                # DVE can read only one PSUM operand per op: stage the
                # broadcast through SBUF (alternate ACT/DVE to balance load)
                bc_sb = elup.tile([128, TB], BF16, name="bc_sb", tag="bc_sb")
                if j % 2 == 0:
                    nc.scalar.copy(bc_sb[:], bps[:])
                else:
                    nc.vector.tensor_copy(bc_sb[:], bps[:])
                nc.vector.tensor_mul(
                    attn_pp[:, half + j * TB : half + (j + 1) * TB],
                    pps[:],
                    bc_sb[:],
                )

        def proj_emit(n):
            # output projection for this chunk: y[o, tok] = Wp^T @ attn + bp
            half = (n % 2) * KC * TB
            for m in range(KC):
                ops_ = psum.tile([128, TB], F32, name="ops", tag="mm")
                for j in range(KC):
                    nc.tensor.matmul(
                        ops_[:],
                        lhsT=wp_sb[:, j * C + m * 128 : j * C + (m + 1) * 128],
                        rhs=attn_pp[:, half + j * TB : half + (j + 1) * TB],
                        start=(j == 0),
                        stop=(j == KC - 1),
                    )
                o_slice = o_pp[:, half + m * TB : half + (m + 1) * TB]
                nc.scalar.activation(
                    o_slice,
                    ops_[:],
                    AF.Identity,
                    bias=bgbp_sb[:, KC + m : KC + m + 1],
                    scale=1.0,
                )
                nc.sync.dma_start(
                    y[128 * m : 128 * (m + 1), n * TB : (n + 1) * TB], o_slice
                )

        attn_emit(0)
        attn_emit(1)
        proj_emit(0)
        attn_emit(2)
        proj_emit(1)
        attn_emit(3)
        proj_emit(2)
        proj_emit(3)


_NC_CACHE = {}


def get_nc():
    if "nc" not in _NC_CACHE:
        _NC_CACHE["nc"] = build_nc()
    return _NC_CACHE["nc"]


def make_in_maps(x, Wqkv, Wg, bg, Wp, bp):
    bf = ml_dtypes.bfloat16
    x = np.asarray(x, dtype=np.float32)
    Wqkv = np.asarray(Wqkv, dtype=np.float32)
    Wg = np.asarray(Wg, dtype=np.float32)
    bg = np.asarray(bg, dtype=np.float32)
    Wp = np.asarray(Wp, dtype=np.float32)
    bp = np.asarray(bp, dtype=np.float32)

    wq = np.ascontiguousarray(Wqkv[:, :C]).astype(bf)
    wkv = np.ascontiguousarray(Wqkv[:, C:]).astype(bf)
    wgt = Wg.astype(bf)
    wp = Wp.astype(bf)
    bgbp = np.concatenate(
        [bg.reshape(KC, 128).T, bp.reshape(KC, 128).T], axis=1
    )
    bgbp = np.ascontiguousarray(bgbp).astype(np.float32)
    e_all = np.zeros((H, C), dtype=bf)
    for h in range(H):
        e_all[h, h * D : (h + 1) * D] = 1.0

    xf = x.reshape(NCORES, T, C)
    in_maps = []
    for c in range(NCORES):
        xtc = np.ascontiguousarray(xf[c].T).astype(bf)
        in_maps.append(
            dict(
                xt=xtc, wq=wq, wkv=wkv, wgt=wgt, wp=wp,
                bgbp=bgbp, e_all=e_all,
            )
        )
    return in_maps


def kernel(x, Wqkv, Wg, bg, Wp, bp, _collect_perf=None):
    nc = get_nc()
    in_maps = make_in_maps(x, Wqkv, Wg, bg, Wp, bp)
    kwargs = {}
    if _collect_perf is not None:
        kwargs = dict(trace=True)
        if _collect_perf.get("tmpdir"):
            kwargs["tmpdir"] = _collect_perf["tmpdir"]
    res = run_bass_kernel_spmd(
        nc, in_maps, core_ids=list(range(NCORES)), **kwargs
    )
    if _collect_perf is not None:
        _collect_perf["exec_time_ns"] = res.exec_time_ns
        _collect_perf["results"] = res
    out = np.empty((NCORES, T, C), dtype=np.float32)
    for c in range(NCORES):
        out[c] = res.results[c]["y"].astype(np.float32).T
    return out.reshape(B, N, C)


# revision 11
# speedup vs baseline: 1.0747x; 1.0747x over previous
"""Gated linear attention on 8 TRN2 NeuronCores.

Sharding: data-parallel over tokens. Core c handles tokens
[c*2048, (c+1)*2048) of the flattened (B*N, C) = (16384, 1024) sequence,
i.e. batch b = c//2, sequence half = c%2. The linear-attention kv state
(and k-sum) needs a reduction over each batch's full sequence, so cores
{2b, 2b+1} all-reduce a small (128, 520) fp32 buffer (kv state + k-sum
for 16 heads) and everything else is local.

Device layouts (per core):
  xt   [C, T]    bf16  x^T shard (host pre-transposes + casts)
  wg   [C, C]    bf16  Wg   (lhsT col-blocks for gate-proj, out [g, tok])
  wkv  [C, 2C]   bf16  Wqkv[:, C:3C]  (rhs for k/v-proj, out [tok, feat])
  wq   [C, C]    bf16  Wqkv[:, :C]    (lhsT for q-proj, out [d, tok])
  wp   [C, C]    bf16  Wp             (lhsT for out-proj, out [o, tok])
  y    [C, T]    bf16  output^T (host transposes back + casts fp32)

q and gates are computed feature-major ([feat, tok]); k and v token-major
([tok, feat]) so the kv einsum can contract over tokens on the partition
axis. Gates are transposed on the DMA xbar (bf16 128x128 tiles) for the
k side. elu(z)+1 is computed as min(exp(z),1) + max(z,0), with exp and
relu on ACT and the mul/combine on DVE so neither engine gates the PE.

v2 scheduling (vs the first working version):
 - gates use k-innermost groups over (m-pair x n-set) PSUM banks so the
   first matmul only needs wg[k=0] + xt[k=0] (768KB) instead of the full
   6MB -> startup stall ~3us instead of ~19us.
 - one 8-buffer PSUM pool; all psum tiles are [128,512]f32 (1 bank).
 - elu combine on GpSimd in the k/v phase (DVE was saturated), on DVE in
   the q phase (GpSimd queue must stay clear for the collective trigger).
 - einsum for 512-token group g is emitted after kvproj chunk 4g+4 so its
   lhsT (DVE/GpSimd products) are ready when the PE reaches it.
 - attention for chunk n+1 is emitted before projection of chunk n, and
   output staging is a ping-pong buffer so the final ACT->DMA chain never
   serializes against DMA completion (this removed a ~23us tail).
"""

import numpy as np
import ml_dtypes

import concourse.bass as bass
import concourse.bacc as bacc
import concourse.tile as tile
import concourse.mybir as mybir
from concourse.bass_utils import run_bass_kernel_spmd

F32 = mybir.dt.float32
BF16 = mybir.dt.bfloat16
AF = mybir.ActivationFunctionType
ALU = mybir.AluOpType

B, N, C = 4, 4096, 1024
H, D = 16, 64
NCORES = 8
T = B * N // NCORES          # 2048 tokens per core
KC = C // 128                # 8 contraction chunks
TB = 512                     # token tile (free dim)
NT = T // TB                 # 4 token tiles
NS = T // 128                # 16 token subchunks (partition-dim tiles)
C2 = 2 * C

REPLICA_GROUPS = [[0, 1], [2, 3], [4, 5], [6, 7]]


def build_nc():
    nc = bacc.Bacc(
        "TRN2", target_bir_lowering=False, debug=False, num_devices=NCORES
    )
    xt = nc.dram_tensor("xt", [C, T], BF16, kind="ExternalInput")
    wq = nc.dram_tensor("wq", [C, C], BF16, kind="ExternalInput")
    wkv = nc.dram_tensor("wkv", [C, C2], BF16, kind="ExternalInput")
    wgt = nc.dram_tensor("wgt", [C, C], BF16, kind="ExternalInput")
    wp = nc.dram_tensor("wp", [C, C], BF16, kind="ExternalInput")
    bgbp = nc.dram_tensor("bgbp", [128, 2 * KC], F32, kind="ExternalInput")
    e_all = nc.dram_tensor("e_all", [H, C], BF16, kind="ExternalInput")
    y = nc.dram_tensor("y", [C, T], BF16, kind="ExternalOutput")

    with tile.TileContext(nc) as tc:
        build_body(nc, tc, xt, wq, wkv, wgt, wp, bgbp, e_all, y)

    nc.compile()
    return nc


def build_body(nc, tc, xt, wq, wkv, wgt, wp, bgbp, e_all, y):
    from contextlib import ExitStack

    with ExitStack() as st:
        constp = st.enter_context(tc.tile_pool(name="constp", bufs=1))
        wbig = st.enter_context(tc.tile_pool(name="wbig", bufs=1))
        wsmall = st.enter_context(tc.tile_pool(name="wsmall", bufs=2))
        big1 = st.enter_context(tc.tile_pool(name="big1", bufs=1))
        gatesp = st.enter_context(tc.tile_pool(name="gatesp", bufs=1))
        qp = st.enter_context(tc.tile_pool(name="qp", bufs=1))
        workp = st.enter_context(tc.tile_pool(name="workp", bufs=2))
        elup = st.enter_context(tc.tile_pool(name="elup", bufs=2))
        psum = st.enter_context(tc.tile_pool(name="psum", bufs=8, space="PSUM"))
        dramp = st.enter_context(tc.tile_pool(name="dramp", bufs=1, space="DRAM"))

        # ---------------------------------------------- initial loads
        # emission order == sync-queue issue order; gates consume wg/xt
        # chunk pairs k-innermost, so interleave them for tight pacing.
        # bgbp goes first (the first sigmoid needs it and a late emission
        # stalls behind the big loads via DMA-semaphore recycling); wkv is
        # deferred into the gates loop so its 6MB doesn't steal wire
        # bandwidth from the wg/xt chunks the gates are waiting on.
        bgbp_sb = constp.tile([128, 2 * KC], F32, name="bgbp_sb")
        nc.sync.dma_start(bgbp_sb[:], bgbp[:])
        wg_sb = wsmall.tile([128, KC * C], BF16, name="wg_sb", tag="wsmall")
        xt_sb = big1.tile([128, KC * T], BF16, name="xt_sb", tag="big1")
        wkv_sb = wbig.tile([128, KC * C2], BF16, name="wkv_sb", tag="wbig")
        e_sb = constp.tile([H, C], BF16, name="e_sb")
        for k in range(KC):
            nc.sync.dma_start(
                wg_sb[:, k * C : (k + 1) * C], wgt[k * 128 : (k + 1) * 128, :]
            )
            nc.sync.dma_start(
                xt_sb[:, k * T : (k + 1) * T], xt[k * 128 : (k + 1) * 128, :]
            )
            if k == 0:
                nc.sync.dma_start(e_sb[:], e_all[:])

        # ---------------------------------------------- phase 1: gates
        # gates[g, tok] = sigmoid(x @ Wg + bg)^T, feature-major.
        # Groups hold (m, n) PSUM banks across the k-innermost accumulation;
        # G0 spans all 4 token tiles for m-pair 0 (8 banks) to maximize
        # per-k-chunk work while the wg/xt DMA stream is still arriving.
        gates_sb = gatesp.tile([128, KC * T], BF16, name="gates_sb", tag="gates")
        gT_full = qp.tile([128, KC * T], BF16, name="gT_full", tag="gT_full")
        gT4 = gT_full.rearrange("p (m s c) -> p m s c", s=NS, c=128)

        groups = [
            (0, (0, 1, 2, 3)),
            (1, (0, 1)), (2, (0, 1)), (3, (0, 1)),
            (1, (2, 3)), (2, (2, 3)), (3, (2, 3)),
        ]
        for gi, (mp, ns) in enumerate(groups):
            ms = (2 * mp, 2 * mp + 1)
            tiles = {}
            for m in ms:
                for n in ns:
                    tiles[(m, n)] = psum.tile(
                        [128, TB], F32, name=f"gps{m}_{n}", tag="mm"
                    )
            for k in range(KC):
                for m in ms:
                    lhsT = wg_sb[:, k * C + m * 128 : k * C + (m + 1) * 128]
                    for n in ns:
                        nc.tensor.matmul(
                            tiles[(m, n)][:],
                            lhsT=lhsT,
                            rhs=xt_sb[:, k * T + n * TB : k * T + (n + 1) * TB],
                            start=(k == 0),
                            stop=(k == KC - 1),
                        )
            for m in ms:
                for n in ns:
                    nc.scalar.activation(
                        gates_sb[:, m * T + n * TB : m * T + (n + 1) * TB],
                        tiles[(m, n)][:],
                        AF.Sigmoid,
                        bias=bgbp_sb[:, m : m + 1],
                        scale=1.0,
                    )
            # transpose the finished half-rows (or full rows for G0) on the
            # DMA xbar: gT[p, m*T + s*128 + c] = gates[m*128 + c, s*128 + p]
            span = len(ns) * TB
            base = ns[0] * TB
            for m in ms:
                nc.sync.dma_start(
                    gT_full[:, m * T + base : m * T + base + span].rearrange(
                        "p (s c) -> p s c", c=128
                    ),
                    gates_sb[:, m * T + base : m * T + base + span],
                    transpose=True,
                )
            # stream wkv in behind the gates groups (2 chunks per group)
            if gi < 4:
                for k in (2 * gi, 2 * gi + 1):
                    nc.sync.dma_start(
                        wkv_sb[:, k * C2 : (k + 1) * C2],
                        wkv[k * 128 : (k + 1) * 128, :],
                    )
            elif gi == 4:
                wq_sb = wsmall.tile([128, KC * C], BF16, name="wq_sb", tag="wsmall")
                nc.sync.dma_start(
                    wq_sb.rearrange("p (k n) -> p k n", k=KC),
                    wq.rearrange("(k p) n -> p k n", p=128),
                )

        # wp into the wsmall slot vacated by wg (waits for last gate matmul)
        wp_sb = wsmall.tile([128, KC * C], BF16, name="wp_sb", tag="wsmall")
        nc.sync.dma_start(
            wp_sb.rearrange("p (k n) -> p k n", k=KC),
            wp.rearrange("(k p) n -> p k n", p=128),
        )

        # ---------------------------------------------- phase 2: k/v + kv state
        # kv_acc block p = cols [130p, 130p+130):
        #   rows 0:64,  cols +0:65   = kv_aug head 2p   (col 64 = k_sum)
        #   rows 64:128, cols +65:130 = kv_aug head 2p+1 (col 129 = k_sum)
        kv_acc = constp.tile([128, KC * 130], F32, name="kv_acc")
        kbfs, vaugs = {}, {}

        def kvproj(s):
            kvps = [
                psum.tile([128, TB], F32, name=f"kvps{n}", tag="mm")
                for n in range(4)
            ]
            for k in range(KC):
                lhsT = xt_sb[:, k * T + s * 128 : k * T + (s + 1) * 128]
                for n in range(4):
                    nc.tensor.matmul(
                        kvps[n][:],
                        lhsT=lhsT,
                        rhs=wkv_sb[:, k * C2 + n * TB : k * C2 + (n + 1) * TB],
                        start=(k == 0),
                        stop=(k == KC - 1),
                    )
            # k = elu(k_raw * g) + 1 = min(exp(kg),1) + max(kg,0)
            k_bf = workp.tile([128, C], BF16, name="k_bf", tag="k_bf", bufs=5)
            for n in range(2):
                kg = elup.tile([128, TB], BF16, name="kg", tag="kg")
                nc.vector.tensor_mul(
                    kg.rearrange("p (m c) -> p m c", c=128),
                    kvps[n].rearrange("p (m c) -> p m c", c=128),
                    gT4[:, 4 * n : 4 * n + 4, s, :],
                )
                relu = elup.tile([128, TB], BF16, name="relu", tag="relu")
                nc.scalar.activation(relu[:], kg[:], AF.Relu)
                ex = elup.tile([128, TB], BF16, name="ex", tag="ex")
                nc.scalar.activation(ex[:], kg[:], AF.Exp)
                nc.vector.scalar_tensor_tensor(
                    k_bf[:, n * TB : (n + 1) * TB],
                    in0=ex[:],
                    scalar=1.0,
                    in1=relu[:],
                    op0=ALU.min,
                    op1=ALU.add,
                )
            # v, augmented with ones column per head (yields k_sum)
            v_aug = workp.tile(
                [128, H * 65], BF16, name="v_aug", tag="v_aug", bufs=5
            )
            v3 = v_aug.rearrange("p (h e) -> p h e", e=65)
            nc.vector.memset(v3[:, :, 64:65], 1.0)
            for n in range(2, 4):
                h0 = (n - 2) * 8
                nc.vector.tensor_copy(
                    v3[:, h0 : h0 + 8, 0:64],
                    kvps[n].rearrange("p (h e) -> p h e", e=64),
                )
            kbfs[s] = k_bf
            vaugs[s] = v_aug

        def einsum(g):
            for p in range(KC):
                eps = psum.tile([128, TB], F32, name="eps", tag="mm")
                for si in range(4):
                    s = g * 4 + si
                    nc.tensor.matmul(
                        eps[:, 0:130],
                        lhsT=kbfs[s][:, 128 * p : 128 * (p + 1)],
                        rhs=vaugs[s][:, 130 * p : 130 * (p + 1)],
                        start=(si == 0),
                        stop=(si == 3),
                    )
                if g == 0:
                    nc.vector.tensor_copy(
                        kv_acc[:, 130 * p : 130 * (p + 1)], eps[:, 0:130]
                    )
                else:
                    nc.vector.tensor_add(
                        kv_acc[:, 130 * p : 130 * (p + 1)],
                        kv_acc[:, 130 * p : 130 * (p + 1)],
                        eps[:, 0:130],
                    )

        # einsum g is emitted after kvproj 4g+4 so the PE never waits on the
        # elu chain of the group's last chunk.
        for s in range(NS):
            kvproj(s)
            if s in (5, 9, 13):
                einsum((s - 5) // 4)
        einsum(3)

        # ---------------------------------------------- kv all-reduce (pairs)
        # compact to [128, 8*65]: head 2p at [0:64, 65p:65p+65],
        # head 2p+1 at [64:128, 65p:65p+65]
        kv_cat = constp.tile([128, KC * 65], F32, name="kv_cat", tag="kv_cat")
        nc.vector.tensor_copy(
            kv_cat[0:64, :].rearrange("p (j e) -> p j e", e=65),
            kv_acc[0:64, :].rearrange("p (j q) -> p j q", q=130)[:, :, 0:65],
        )
        nc.vector.tensor_copy(
            kv_cat[64:128, :].rearrange("p (j e) -> p j e", e=65),
            kv_acc[64:128, :].rearrange("p (j q) -> p j q", q=130)[:, :, 65:130],
        )
        bounce_in = dramp.tile([128, KC * 65], F32, name="bounce_in")
        bounce_out = dramp.tile([128, KC * 65], F32, name="bounce_out")
        nc.sync.dma_start(bounce_in[:], kv_cat[:])
        nc.gpsimd.collective_compute(
            "AllReduce",
            ALU.add,
            replica_groups=REPLICA_GROUPS,
            ins=[bounce_in.opt()],
            outs=[bounce_out.opt()],
        )

        # ---------------------------------------------- phase 3: q (overlaps AR)
        # q feature-major in (m-pair x n-pair) 4-bank groups, k innermost.
        # elu combine stays on DVE here: the gpsimd queue holds the
        # collective trigger and must not be backed up behind q's work.
        q_sb = wbig.tile([128, KC * T], BF16, name="q_sb", tag="wbig")
        bds, blks, rbs = [], [], [None] * NT

        def q_group(mp, np_):
            ms = (2 * mp, 2 * mp + 1)
            ns = (2 * np_, 2 * np_ + 1)
            tiles = {}
            for m in ms:
                for n in ns:
                    tiles[(m, n)] = psum.tile(
                        [128, TB], F32, name=f"qps{m}_{n}", tag="mm"
                    )
            for k in range(KC):
                for m in ms:
                    lhsT = wq_sb[:, k * C + m * 128 : k * C + (m + 1) * 128]
                    for n in ns:
                        nc.tensor.matmul(
                            tiles[(m, n)][:],
                            lhsT=lhsT,
                            rhs=xt_sb[:, k * T + n * TB : k * T + (n + 1) * TB],
                            start=(k == 0),
                            stop=(k == KC - 1),
                        )
            for m in ms:
                for n in ns:
                    qg = elup.tile([128, TB], BF16, name="qg", tag="kg")
                    nc.vector.tensor_mul(
                        qg[:],
                        tiles[(m, n)][:],
                        gates_sb[:, m * T + n * TB : m * T + (n + 1) * TB],
                    )
                    relu = elup.tile([128, TB], BF16, name="relu2", tag="relu")
                    nc.scalar.activation(relu[:], qg[:], AF.Relu)
                    ex = elup.tile([128, TB], BF16, name="ex2", tag="ex")
                    nc.scalar.activation(ex[:], qg[:], AF.Exp)
                    nc.vector.scalar_tensor_tensor(
                        q_sb[:, m * T + n * TB : m * T + (n + 1) * TB],
                        in0=ex[:],
                        scalar=1.0,
                        in1=relu[:],
                        op0=ALU.min,
                        op1=ALU.add,
                    )

        def ar_result_prep():
            # AR result -> bf16 block-diagonal kv / k_sum tiles. Emitted in
            # the middle of the q phase so the DVE chain runs under q matmuls
            # instead of stalling the PE at the q -> attention boundary.
            kv_f32 = constp.tile([128, KC * 65], F32, name="kv_f32", tag="kv_cat")
            nc.sync.dma_start(kv_f32[:], bounce_out[:])
            kv_bf = constp.tile([128, KC * 65], BF16, name="kv_bf")
            nc.vector.tensor_copy(kv_bf[:], kv_f32[:])
            for j in range(KC):
                bd = constp.tile([128, 128], BF16, name=f"bd{j}")
                nc.vector.memset(bd[:], 0.0)
                nc.vector.tensor_copy(
                    bd[0:64, 0:64], kv_bf[0:64, 65 * j : 65 * j + 64]
                )
                nc.vector.tensor_copy(
                    bd[64:128, 64:128], kv_bf[64:128, 65 * j : 65 * j + 64]
                )
                bds.append(bd)
            for j in range(KC):
                bj = constp.tile([128, H], BF16, name=f"blk{j}")
                nc.vector.memset(bj[:], 0.0)
                for par in range(2):
                    h = 2 * j + par
                    nc.vector.tensor_copy(
                        bj[par * 64 : (par + 1) * 64, h : h + 1],
                        kv_bf[par * 64 : (par + 1) * 64, 65 * j + 64 : 65 * j + 65],
                    )
                blks.append(bj)

        def norm_emit(n):
            # normalizer reciprocal for token chunk n
            nps = psum.tile([128, TB], F32, name="nps", tag="mm")
            for j in range(KC):
                nc.tensor.matmul(
                    nps[0:H, :],
                    lhsT=blks[j][:],
                    rhs=q_sb[:, j * T + n * TB : j * T + (n + 1) * TB],
                    start=(j == 0),
                    stop=(j == KC - 1),
                )
            nc.vector.tensor_scalar_add(nps[0:H, :], nps[0:H, :], 1e-8)
            nrec = elup.tile([H, TB], F32, name="nrec", tag="nrec")
            nc.vector.reciprocal_approx_fast(nrec[:], nps[0:H, :])
            rb = constp.tile([H, TB], BF16, name=f"rb{n}")
            nc.vector.tensor_copy(rb[:], nrec[:])
            rbs[n] = rb

        # token-pair-outer so q for n=0,1 completes after 4 groups; the AR
        # prep and first normalizers then hide under the remaining q groups.
        for mp in range(4):
            q_group(mp, 0)
        ar_result_prep()
        norm_emit(0)
        norm_emit(1)
        for mp in range(4):
            q_group(mp, 1)
        norm_emit(2)
        norm_emit(3)

        # ping-pong buffers: attention output (feature-major) and staged y
        attn_pp = big1.tile([128, 2 * KC * TB], BF16, name="attn_pp", tag="big1")
        o_pp = gatesp.tile([128, 2 * KC * TB], BF16, name="o_pp", tag="gates")

        def attn_emit(n):
            # attn[e, tok] = (q @ kv) * bcast(recip)   (feature-major)
            half = (n % 2) * KC * TB
            for j in range(KC):
                pps = psum.tile([128, TB], F32, name="pps", tag="mm")
                nc.tensor.matmul(
                    pps[:],
                    lhsT=bds[j][:],
                    rhs=q_sb[:, j * T + n * TB : j * T + (n + 1) * TB],
                    start=True,
                    stop=True,
                )
                bps = psum.tile([128, TB], F32, name="bps", tag="mm")
                nc.tensor.matmul(
                    bps[:],
                    lhsT=e_sb[:, j * 128 : (j + 1) * 128],
                    rhs=rbs[n][:],
                    start=True,
                    stop=True,
                )
                # DVE can read only one PSUM operand per op: stage the
                # broadcast through SBUF (alternate ACT/DVE to balance load)
                bc_sb = elup.tile([128, TB], BF16, name="bc_sb", tag="bc_sb")
                if j % 2 == 0:
                    nc.scalar.copy(bc_sb[:], bps[:])
                else:
                    nc.vector.tensor_copy(bc_sb[:], bps[:])
                nc.vector.tensor_mul(
                    attn_pp[:, half + j * TB : half + (j + 1) * TB],
                    pps[:],
                    bc_sb[:],
                )

        def proj_emit(n):
            # output projection for this chunk: y[o, tok] = Wp^T @ attn + bp
            half = (n % 2) * KC * TB
            for m in range(KC):
                ops_ = psum.tile([128, TB], F32, name="ops", tag="mm")
                for j in range(KC):
                    nc.tensor.matmul(
                        ops_[:],
                        lhsT=wp_sb[:, j * C + m * 128 : j * C + (m + 1) * 128],
                        rhs=attn_pp[:, half + j * TB : half + (j + 1) * TB],
                        start=(j == 0),
                        stop=(j == KC - 1),
                    )
                o_slice = o_pp[:, half + m * TB : half + (m + 1) * TB]
                nc.scalar.activation(
                    o_slice,
                    ops_[:],
                    AF.Identity,
                    bias=bgbp_sb[:, KC + m : KC + m + 1],
                    scale=1.0,
                )
                nc.sync.dma_start(
                    y[128 * m : 128 * (m + 1), n * TB : (n + 1) * TB], o_slice
                )

        attn_emit(0)
        attn_emit(1)
        proj_emit(0)
        attn_emit(2)
        proj_emit(1)
        attn_emit(3)
        proj_emit(2)
        proj_emit(3)


_NC_CACHE = {}


def get_nc():
    if "nc" not in _NC_CACHE:
        _NC_CACHE["nc"] = build_nc()
    return _NC_CACHE["nc"]


def make_in_maps(x, Wqkv, Wg, bg, Wp, bp):
    bf = ml_dtypes.bfloat16
    x = np.asarray(x, dtype=np.float32)
    Wqkv = np.asarray(Wqkv, dtype=np.float32)
    Wg = np.asarray(Wg, dtype=np.float32)
    bg = np.asarray(bg, dtype=np.float32)
    Wp = np.asarray(Wp, dtype=np.float32)
    bp = np.asarray(bp, dtype=np.float32)

    wq = np.ascontiguousarray(Wqkv[:, :C]).astype(bf)
    wkv = np.ascontiguousarray(Wqkv[:, C:]).astype(bf)
    wgt = Wg.astype(bf)
    wp = Wp.astype(bf)
    bgbp = np.concatenate(
        [bg.reshape(KC, 128).T, bp.reshape(KC, 128).T], axis=1
    )
    bgbp = np.ascontiguousarray(bgbp).astype(np.float32)
    e_all = np.zeros((H, C), dtype=bf)
    for h in range(H):
        e_all[h, h * D : (h + 1) * D] = 1.0

    xf = x.reshape(NCORES, T, C)
    in_maps = []
    for c in range(NCORES):
        xtc = np.ascontiguousarray(xf[c].T).astype(bf)
        in_maps.append(
            dict(
                xt=xtc, wq=wq, wkv=wkv, wgt=wgt, wp=wp,
                bgbp=bgbp, e_all=e_all,
            )
        )
    return in_maps


def kernel(x, Wqkv, Wg, bg, Wp, bp, _collect_perf=None):
    nc = get_nc()
    in_maps = make_in_maps(x, Wqkv, Wg, bg, Wp, bp)
    kwargs = {}
    if _collect_perf is not None:
        kwargs = dict(trace=True)
        if _collect_perf.get("tmpdir"):
            kwargs["tmpdir"] = _collect_perf["tmpdir"]
    res = run_bass_kernel_spmd(
        nc, in_maps, core_ids=list(range(NCORES)), **kwargs
    )
    if _collect_perf is not None:
        _collect_perf["exec_time_ns"] = res.exec_time_ns
        _collect_perf["results"] = res
    out = np.empty((NCORES, T, C), dtype=np.float32)
    for c in range(NCORES):
        out[c] = res.results[c]["y"].astype(np.float32).T
    return out.reshape(B, N, C)


# revision 15
# speedup vs baseline: 1.1305x; 1.0519x over previous
"""Gated linear attention on 8 TRN2 NeuronCores.

Sharding: data-parallel over tokens. Core c handles tokens
[c*2048, (c+1)*2048) of the flattened (B*N, C) = (16384, 1024) sequence,
i.e. batch b = c//2, sequence half = c%2. The linear-attention kv state
(and k-sum) needs a reduction over each batch's full sequence, so cores
{2b, 2b+1} all-reduce a small (128, 520) fp32 buffer (kv state + k-sum
for 16 heads) and everything else is local.

Device layouts (per core):
  xt   [C, T]    bf16  x^T shard (host pre-transposes + casts)
  wg   [C, C]    bf16  Wg   (lhsT col-blocks for gate-proj, out [g, tok])
  wkv  [C, 2C]   bf16  Wqkv[:, C:3C]  (rhs for k/v-proj, out [tok, feat])
  wq   [C, C]    bf16  Wqkv[:, :C]    (lhsT for q-proj, out [d, tok])
  wp   [C, C]    bf16  Wp             (lhsT for out-proj, out [o, tok])
  y    [C, T]    bf16  output^T (host transposes back + casts fp32)

q and gates are computed feature-major ([feat, tok]); k and v token-major
([tok, feat]) so the kv einsum can contract over tokens on the partition
axis. Gates are transposed on the DMA xbar (bf16 128x128 tiles) for the
k side. elu(z)+1 is computed as min(exp(z),1) + max(z,0), with exp and
relu on ACT and the mul/combine on DVE so neither engine gates the PE.

v2 scheduling (vs the first working version):
 - gates use k-innermost groups over (m-pair x n-set) PSUM banks so the
   first matmul only needs wg[k=0] + xt[k=0] (768KB) instead of the full
   6MB -> startup stall ~3us instead of ~19us.
 - one 8-buffer PSUM pool; all psum tiles are [128,512]f32 (1 bank).
 - elu combine on GpSimd in the k/v phase (DVE was saturated), on DVE in
   the q phase (GpSimd queue must stay clear for the collective trigger).
 - einsum for 512-token group g is emitted after kvproj chunk 4g+4 so its
   lhsT (DVE/GpSimd products) are ready when the PE reaches it.
 - attention for chunk n+1 is emitted before projection of chunk n, and
   output staging is a ping-pong buffer so the final ACT->DMA chain never
   serializes against DMA completion (this removed a ~23us tail).
"""

import numpy as np
import ml_dtypes

import concourse.bass as bass
import concourse.bacc as bacc
import concourse.tile as tile
import concourse.mybir as mybir
from concourse.bass_utils import run_bass_kernel_spmd

F32 = mybir.dt.float32
BF16 = mybir.dt.bfloat16
AF = mybir.ActivationFunctionType
ALU = mybir.AluOpType

B, N, C = 4, 4096, 1024
H, D = 16, 64
NCORES = 8
T = B * N // NCORES          # 2048 tokens per core
KC = C // 128                # 8 contraction chunks
TB = 512                     # token tile (free dim)
NT = T // TB                 # 4 token tiles
NS = T // 128                # 16 token subchunks (partition-dim tiles)
C2 = 2 * C

REPLICA_GROUPS = [[0, 1], [2, 3], [4, 5], [6, 7]]


def build_nc():
    nc = bacc.Bacc(
        "TRN2", target_bir_lowering=False, debug=False, num_devices=NCORES
    )
    xt = nc.dram_tensor("xt", [C, T], BF16, kind="ExternalInput")
    wq = nc.dram_tensor("wq", [C, C], BF16, kind="ExternalInput")
    wkv = nc.dram_tensor("wkv", [C, C2], BF16, kind="ExternalInput")
    wgt = nc.dram_tensor("wgt", [C, C], BF16, kind="ExternalInput")
    wp = nc.dram_tensor("wp", [C, C], BF16, kind="ExternalInput")
    bgbp = nc.dram_tensor("bgbp", [128, 2 * KC], F32, kind="ExternalInput")
    e_all = nc.dram_tensor("e_all", [H, C], BF16, kind="ExternalInput")
    y = nc.dram_tensor("y", [C, T], BF16, kind="ExternalOutput")

    with tile.TileContext(nc) as tc:
        build_body(nc, tc, xt, wq, wkv, wgt, wp, bgbp, e_all, y)

    nc.compile()
    return nc


def build_body(nc, tc, xt, wq, wkv, wgt, wp, bgbp, e_all, y):
    from contextlib import ExitStack

    with ExitStack() as st:
        constp = st.enter_context(tc.tile_pool(name="constp", bufs=1))
        wbig = st.enter_context(tc.tile_pool(name="wbig", bufs=1))
        wsmall = st.enter_context(tc.tile_pool(name="wsmall", bufs=2))
        big1 = st.enter_context(tc.tile_pool(name="big1", bufs=1))
        gatesp = st.enter_context(tc.tile_pool(name="gatesp", bufs=1))
        qp = st.enter_context(tc.tile_pool(name="qp", bufs=1))
        workp = st.enter_context(tc.tile_pool(name="workp", bufs=2))
        elup = st.enter_context(tc.tile_pool(name="elup", bufs=2))
        psum = st.enter_context(tc.tile_pool(name="psum", bufs=8, space="PSUM"))
        dramp = st.enter_context(tc.tile_pool(name="dramp", bufs=1, space="DRAM"))

        # ---------------------------------------------- initial loads
        # emission order == sync-queue issue order; gates consume wg/xt
        # chunk pairs k-innermost, so interleave them for tight pacing.
        # bgbp goes first (the first sigmoid needs it and a late emission
        # stalls behind the big loads via DMA-semaphore recycling); wkv is
        # deferred into the gates loop so its 6MB doesn't steal wire
        # bandwidth from the wg/xt chunks the gates are waiting on.
        bgbp_sb = constp.tile([128, 2 * KC], F32, name="bgbp_sb")
        nc.sync.dma_start(bgbp_sb[:], bgbp[:])
        wg_sb = wsmall.tile([128, KC * C], BF16, name="wg_sb", tag="wsmall")
        xt_sb = big1.tile([128, KC * T], BF16, name="xt_sb", tag="big1")
        wkv_sb = wbig.tile([128, KC * C2], BF16, name="wkv_sb", tag="wbig")
        e_sb = constp.tile([H, C], BF16, name="e_sb")
        for k in range(KC):
            nc.sync.dma_start(
                wg_sb[:, k * C : (k + 1) * C], wgt[k * 128 : (k + 1) * 128, :]
            )
            nc.sync.dma_start(
                xt_sb[:, k * T : (k + 1) * T], xt[k * 128 : (k + 1) * 128, :]
            )
            if k == 0:
                nc.sync.dma_start(e_sb[:], e_all[:])

        # ---------------------------------------------- phase 1: gates
        # gates[g, tok] = sigmoid(x @ Wg + bg)^T, feature-major.
        # Groups hold (m, n) PSUM banks across the k-innermost accumulation;
        # G0 spans all 4 token tiles for m-pair 0 (8 banks) to maximize
        # per-k-chunk work while the wg/xt DMA stream is still arriving.
        gates_sb = gatesp.tile([128, KC * T], BF16, name="gates_sb", tag="gates")
        gT_full = qp.tile([128, KC * T], BF16, name="gT_full", tag="gT_full")
        gT4 = gT_full.rearrange("p (m s c) -> p m s c", s=NS, c=128)

        groups = [
            (0, (0, 1, 2, 3)),
            (1, (0, 1)), (2, (0, 1)), (3, (0, 1)),
            (1, (2, 3)), (2, (2, 3)), (3, (2, 3)),
        ]
        for gi, (mp, ns) in enumerate(groups):
            ms = (2 * mp, 2 * mp + 1)
            tiles = {}
            for m in ms:
                for n in ns:
                    tiles[(m, n)] = psum.tile(
                        [128, TB], F32, name=f"gps{m}_{n}", tag="mm"
                    )
            for k in range(KC):
                for m in ms:
                    lhsT = wg_sb[:, k * C + m * 128 : k * C + (m + 1) * 128]
                    for n in ns:
                        nc.tensor.matmul(
                            tiles[(m, n)][:],
                            lhsT=lhsT,
                            rhs=xt_sb[:, k * T + n * TB : k * T + (n + 1) * TB],
                            start=(k == 0),
                            stop=(k == KC - 1),
                        )
            for m in ms:
                for n in ns:
                    nc.scalar.activation(
                        gates_sb[:, m * T + n * TB : m * T + (n + 1) * TB],
                        tiles[(m, n)][:],
                        AF.Sigmoid,
                        bias=bgbp_sb[:, m : m + 1],
                        scale=1.0,
                    )
            # transpose the finished half-rows (or full rows for G0) on the
            # DMA xbar: gT[p, m*T + s*128 + c] = gates[m*128 + c, s*128 + p]
            span = len(ns) * TB
            base = ns[0] * TB
            for m in ms:
                nc.sync.dma_start(
                    gT_full[:, m * T + base : m * T + base + span].rearrange(
                        "p (s c) -> p s c", c=128
                    ),
                    gates_sb[:, m * T + base : m * T + base + span],
                    transpose=True,
                )
            # stream wkv in behind the gates groups (2 chunks per group)
            if gi < 4:
                for k in (2 * gi, 2 * gi + 1):
                    nc.sync.dma_start(
                        wkv_sb[:, k * C2 : (k + 1) * C2],
                        wkv[k * 128 : (k + 1) * 128, :],
                    )
            elif gi == 4:
                wq_sb = wsmall.tile([128, KC * C], BF16, name="wq_sb", tag="wsmall")
                nc.sync.dma_start(
                    wq_sb.rearrange("p (k n) -> p k n", k=KC),
                    wq.rearrange("(k p) n -> p k n", p=128),
                )

        # wp into the wsmall slot vacated by wg (waits for last gate matmul)
        wp_sb = wsmall.tile([128, KC * C], BF16, name="wp_sb", tag="wsmall")
        nc.sync.dma_start(
            wp_sb.rearrange("p (k n) -> p k n", k=KC),
            wp.rearrange("(k p) n -> p k n", p=128),
        )

        # ---------------------------------------------- phase 2: k/v + kv state
        # kv_acc block p = cols [130p, 130p+130):
        #   rows 0:64,  cols +0:65   = kv_aug head 2p   (col 64 = k_sum)
        #   rows 64:128, cols +65:130 = kv_aug head 2p+1 (col 129 = k_sum)
        kv_acc = constp.tile([128, KC * 130], F32, name="kv_acc")
        kbfs, vaugs = {}, {}

        def kvproj(s):
            kvps = [
                psum.tile([128, TB], F32, name=f"kvps{n}", tag="mm")
                for n in range(4)
            ]
            for k in range(KC):
                lhsT = xt_sb[:, k * T + s * 128 : k * T + (s + 1) * 128]
                for n in range(4):
                    nc.tensor.matmul(
                        kvps[n][:],
                        lhsT=lhsT,
                        rhs=wkv_sb[:, k * C2 + n * TB : k * C2 + (n + 1) * TB],
                        start=(k == 0),
                        stop=(k == KC - 1),
                    )
            # k = elu(k_raw * g) + 1 = min(exp(kg),1) + max(kg,0)
            k_bf = workp.tile([128, C], BF16, name="k_bf", tag="k_bf", bufs=5)
            for n in range(2):
                kg = elup.tile([128, TB], BF16, name="kg", tag="kg")
                nc.vector.tensor_mul(
                    kg.rearrange("p (m c) -> p m c", c=128),
                    kvps[n].rearrange("p (m c) -> p m c", c=128),
                    gT4[:, 4 * n : 4 * n + 4, s, :],
                )
                relu = elup.tile([128, TB], BF16, name="relu", tag="relu")
                nc.scalar.activation(relu[:], kg[:], AF.Relu)
                ex = elup.tile([128, TB], BF16, name="ex", tag="ex")
                nc.scalar.activation(ex[:], kg[:], AF.Exp)
                nc.vector.scalar_tensor_tensor(
                    k_bf[:, n * TB : (n + 1) * TB],
                    in0=ex[:],
                    scalar=1.0,
                    in1=relu[:],
                    op0=ALU.min,
                    op1=ALU.add,
                )
            # v, augmented with ones column per head (yields k_sum)
            v_aug = workp.tile(
                [128, H * 65], BF16, name="v_aug", tag="v_aug", bufs=5
            )
            v3 = v_aug.rearrange("p (h e) -> p h e", e=65)
            nc.vector.memset(v3[:, :, 64:65], 1.0)
            for n in range(2, 4):
                h0 = (n - 2) * 8
                nc.vector.tensor_copy(
                    v3[:, h0 : h0 + 8, 0:64],
                    kvps[n].rearrange("p (h e) -> p h e", e=64),
                )
            kbfs[s] = k_bf
            vaugs[s] = v_aug

        def einsum(g):
            for p in range(KC):
                eps = psum.tile([128, TB], F32, name="eps", tag="mm")
                for si in range(4):
                    s = g * 4 + si
                    nc.tensor.matmul(
                        eps[:, 0:130],
                        lhsT=kbfs[s][:, 128 * p : 128 * (p + 1)],
                        rhs=vaugs[s][:, 130 * p : 130 * (p + 1)],
                        start=(si == 0),
                        stop=(si == 3),
                    )
                if g == 0:
                    nc.vector.tensor_copy(
                        kv_acc[:, 130 * p : 130 * (p + 1)], eps[:, 0:130]
                    )
                else:
                    nc.vector.tensor_add(
                        kv_acc[:, 130 * p : 130 * (p + 1)],
                        kv_acc[:, 130 * p : 130 * (p + 1)],
                        eps[:, 0:130],
                    )

        # einsum g is emitted after kvproj 4g+4 so the PE never waits on the
        # elu chain of the group's last chunk.
        for s in range(NS):
            kvproj(s)
            if s in (5, 9, 13):
                einsum((s - 5) // 4)
        einsum(3)

        # ---------------------------------------------- kv all-reduce (pairs)
        # compact to [128, 8*65]: head 2p at [0:64, 65p:65p+65],
        # head 2p+1 at [64:128, 65p:65p+65]
        # bf16 payload: halves the wire time, and the pair-sum loses only
        # ~0.4% relative on a tensor that already tolerates bf16 downstream.
        kv_cat = constp.tile([128, KC * 65], BF16, name="kv_cat", tag="kv_cat")
        nc.vector.tensor_copy(
            kv_cat[0:64, :].rearrange("p (j e) -> p j e", e=65),
            kv_acc[0:64, :].rearrange("p (j q) -> p j q", q=130)[:, :, 0:65],
        )
        nc.vector.tensor_copy(
            kv_cat[64:128, :].rearrange("p (j e) -> p j e", e=65),
            kv_acc[64:128, :].rearrange("p (j q) -> p j q", q=130)[:, :, 65:130],
        )
        bounce_in = dramp.tile([128, KC * 65], BF16, name="bounce_in")
        bounce_out = dramp.tile([128, KC * 65], BF16, name="bounce_out")
        nc.sync.dma_start(bounce_in[:], kv_cat[:])
        nc.gpsimd.collective_compute(
            "AllReduce",
            ALU.add,
            replica_groups=REPLICA_GROUPS,
            ins=[bounce_in.opt()],
            outs=[bounce_out.opt()],
        )

        # ---------------------------------------------- phase 3: q (overlaps AR)
        # q feature-major in (m-pair x n-pair) 4-bank groups, k innermost.
        # elu combine stays on DVE here: the gpsimd queue holds the
        # collective trigger and must not be backed up behind q's work.
        q_sb = wbig.tile([128, KC * T], BF16, name="q_sb", tag="wbig")
        bds, blks, rbs = [], [], [None] * NT

        def q_group(mp, np_):
            ms = (2 * mp, 2 * mp + 1)
            ns = (2 * np_, 2 * np_ + 1)
            tiles = {}
            for m in ms:
                for n in ns:
                    tiles[(m, n)] = psum.tile(
                        [128, TB], F32, name=f"qps{m}_{n}", tag="mm"
                    )
            for k in range(KC):
                for m in ms:
                    lhsT = wq_sb[:, k * C + m * 128 : k * C + (m + 1) * 128]
                    for n in ns:
                        nc.tensor.matmul(
                            tiles[(m, n)][:],
                            lhsT=lhsT,
                            rhs=xt_sb[:, k * T + n * TB : k * T + (n + 1) * TB],
                            start=(k == 0),
                            stop=(k == KC - 1),
                        )
            for m in ms:
                for n in ns:
                    qg = elup.tile([128, TB], BF16, name="qg", tag="kg")
                    nc.vector.tensor_mul(
                        qg[:],
                        tiles[(m, n)][:],
                        gates_sb[:, m * T + n * TB : m * T + (n + 1) * TB],
                    )
                    relu = elup.tile([128, TB], BF16, name="relu2", tag="relu")
                    nc.scalar.activation(relu[:], qg[:], AF.Relu)
                    ex = elup.tile([128, TB], BF16, name="ex2", tag="ex")
                    nc.scalar.activation(ex[:], qg[:], AF.Exp)
                    nc.vector.scalar_tensor_tensor(
                        q_sb[:, m * T + n * TB : m * T + (n + 1) * TB],
                        in0=ex[:],
                        scalar=1.0,
                        in1=relu[:],
                        op0=ALU.min,
                        op1=ALU.add,
                    )

        def ar_result_prep():
            # AR result -> bf16 block-diagonal kv / k_sum tiles. Emitted in
            # the middle of the q phase so the DVE chain runs under q matmuls
            # instead of stalling the PE at the q -> attention boundary.
            kv_bf = constp.tile([128, KC * 65], BF16, name="kv_bf", tag="kv_cat")
            nc.sync.dma_start(kv_bf[:], bounce_out[:])
            for j in range(KC):
                bd = constp.tile([128, 128], BF16, name=f"bd{j}")
                nc.vector.memset(bd[:], 0.0)
                nc.vector.tensor_copy(
                    bd[0:64, 0:64], kv_bf[0:64, 65 * j : 65 * j + 64]
                )
                nc.vector.tensor_copy(
                    bd[64:128, 64:128], kv_bf[64:128, 65 * j : 65 * j + 64]
                )
                bds.append(bd)
            for j in range(KC):
                bj = constp.tile([128, H], BF16, name=f"blk{j}")
                nc.vector.memset(bj[:], 0.0)
                for par in range(2):
                    h = 2 * j + par
                    nc.vector.tensor_copy(
                        bj[par * 64 : (par + 1) * 64, h : h + 1],
                        kv_bf[par * 64 : (par + 1) * 64, 65 * j + 64 : 65 * j + 65],
                    )
                blks.append(bj)

        def norm_emit(n):
            # normalizer reciprocal for token chunk n
            nps = psum.tile([128, TB], F32, name="nps", tag="mm")
            for j in range(KC):
                nc.tensor.matmul(
                    nps[0:H, :],
                    lhsT=blks[j][:],
                    rhs=q_sb[:, j * T + n * TB : j * T + (n + 1) * TB],
                    start=(j == 0),
                    stop=(j == KC - 1),
                )
            nc.vector.tensor_scalar_add(nps[0:H, :], nps[0:H, :], 1e-8)
            nrec = elup.tile([H, TB], F32, name="nrec", tag="nrec")
            nc.vector.reciprocal_approx_fast(nrec[:], nps[0:H, :])
            rb = constp.tile([H, TB], BF16, name=f"rb{n}")
            nc.vector.tensor_copy(rb[:], nrec[:])
            rbs[n] = rb

        # AR-result prep (DVE-only) is emitted just before the LAST q group:
        # late enough that the in-order DVE queue reaches it after the AR has
        # completed (so it never blocks earlier q evictions), early enough
        # that bds/blks are ready when the normalizer matmuls start. All
        # normalizer matmuls come after the last q group so the PE queue
        # never parks on the AR semaphore.
        for mp in range(4):
            q_group(mp, 0)
        for mp in range(4):
            if mp == 3:
                ar_result_prep()
            q_group(mp, 1)
        for n in range(NT):
            norm_emit(n)

        # ping-pong buffers: attention output (feature-major) and staged y
        attn_pp = big1.tile([128, 2 * KC * TB], BF16, name="attn_pp", tag="big1")
        o_pp = gatesp.tile([128, 2 * KC * TB], BF16, name="o_pp", tag="gates")

        def attn_emit(n):
            # attn[e, tok] = (q @ kv) * bcast(recip)   (feature-major)
            half = (n % 2) * KC * TB
            for j in range(KC):
                pps = psum.tile([128, TB], F32, name="pps", tag="mm")
                nc.tensor.matmul(
                    pps[:],
                    lhsT=bds[j][:],
                    rhs=q_sb[:, j * T + n * TB : j * T + (n + 1) * TB],
                    start=True,
                    stop=True,
                )
                bps = psum.tile([128, TB], F32, name="bps", tag="mm")
                nc.tensor.matmul(
                    bps[:],
                    lhsT=e_sb[:, j * 128 : (j + 1) * 128],
                    rhs=rbs[n][:],
                    start=True,
                    stop=True,
                )
                # DVE can read only one PSUM operand per op: stage the
                # broadcast through SBUF (alternate ACT/DVE to balance load)
                bc_sb = elup.tile([128, TB], BF16, name="bc_sb", tag="bc_sb")
                if j % 2 == 0:
                    nc.scalar.copy(bc_sb[:], bps[:])
                else:
                    nc.vector.tensor_copy(bc_sb[:], bps[:])
                nc.vector.tensor_mul(
                    attn_pp[:, half + j * TB : half + (j + 1) * TB],
                    pps[:],
                    bc_sb[:],
                )

        def proj_emit(n):
            # output projection for this chunk: y[o, tok] = Wp^T @ attn + bp
            half = (n % 2) * KC * TB
            for m in range(KC):
                ops_ = psum.tile([128, TB], F32, name="ops", tag="mm")
                for j in range(KC):
                    nc.tensor.matmul(
                        ops_[:],
                        lhsT=wp_sb[:, j * C + m * 128 : j * C + (m + 1) * 128],
                        rhs=attn_pp[:, half + j * TB : half + (j + 1) * TB],
                        start=(j == 0),
                        stop=(j == KC - 1),
                    )
                o_slice = o_pp[:, half + m * TB : half + (m + 1) * TB]
                nc.scalar.activation(
                    o_slice,
                    ops_[:],
                    AF.Identity,
                    bias=bgbp_sb[:, KC + m : KC + m + 1],
                    scale=1.0,
                )
                nc.sync.dma_start(
                    y[128 * m : 128 * (m + 1), n * TB : (n + 1) * TB], o_slice
                )

        attn_emit(0)
        attn_emit(1)
        proj_emit(0)
        attn_emit(2)
        proj_emit(1)
        attn_emit(3)
        proj_emit(2)
        proj_emit(3)


_NC_CACHE = {}


def get_nc():
    if "nc" not in _NC_CACHE:
        _NC_CACHE["nc"] = build_nc()
    return _NC_CACHE["nc"]


def make_in_maps(x, Wqkv, Wg, bg, Wp, bp):
    bf = ml_dtypes.bfloat16
    x = np.asarray(x, dtype=np.float32)
    Wqkv = np.asarray(Wqkv, dtype=np.float32)
    Wg = np.asarray(Wg, dtype=np.float32)
    bg = np.asarray(bg, dtype=np.float32)
    Wp = np.asarray(Wp, dtype=np.float32)
    bp = np.asarray(bp, dtype=np.float32)

    wq = np.ascontiguousarray(Wqkv[:, :C]).astype(bf)
    wkv = np.ascontiguousarray(Wqkv[:, C:]).astype(bf)
    wgt = Wg.astype(bf)
    wp = Wp.astype(bf)
    bgbp = np.concatenate(
        [bg.reshape(KC, 128).T, bp.reshape(KC, 128).T], axis=1
    )
    bgbp = np.ascontiguousarray(bgbp).astype(np.float32)
    e_all = np.zeros((H, C), dtype=bf)
    for h in range(H):
        e_all[h, h * D : (h + 1) * D] = 1.0

    xf = x.reshape(NCORES, T, C)
    in_maps = []
    for c in range(NCORES):
        xtc = np.ascontiguousarray(xf[c].T).astype(bf)
        in_maps.append(
            dict(
                xt=xtc, wq=wq, wkv=wkv, wgt=wgt, wp=wp,
                bgbp=bgbp, e_all=e_all,
            )
        )
    return in_maps


def kernel(x, Wqkv, Wg, bg, Wp, bp, _collect_perf=None):
    nc = get_nc()
    in_maps = make_in_maps(x, Wqkv, Wg, bg, Wp, bp)
    kwargs = {}
    if _collect_perf is not None:
        kwargs = dict(trace=True)
        if _collect_perf.get("tmpdir"):
            kwargs["tmpdir"] = _collect_perf["tmpdir"]
    res = run_bass_kernel_spmd(
        nc, in_maps, core_ids=list(range(NCORES)), **kwargs
    )
    if _collect_perf is not None:
        _collect_perf["exec_time_ns"] = res.exec_time_ns
        _collect_perf["results"] = res
    out = np.empty((NCORES, T, C), dtype=np.float32)
    for c in range(NCORES):
        out[c] = res.results[c]["y"].astype(np.float32).T
    return out.reshape(B, N, C)


# revision 22
# speedup vs baseline: 1.1325x; 1.0018x over previous
"""Gated linear attention on 8 TRN2 NeuronCores.

Sharding: data-parallel over tokens. Core c handles tokens
[c*2048, (c+1)*2048) of the flattened (B*N, C) = (16384, 1024) sequence,
i.e. batch b = c//2, sequence half = c%2. The linear-attention kv state
(and k-sum) needs a reduction over each batch's full sequence, so cores
{2b, 2b+1} all-reduce a small (128, 520) fp32 buffer (kv state + k-sum
for 16 heads) and everything else is local.

Device layouts (per core):
  xt   [C, T]    bf16  x^T shard (host pre-transposes + casts)
  wg   [C, C]    bf16  Wg   (lhsT col-blocks for gate-proj, out [g, tok])
  wkv  [C, 2C]   bf16  Wqkv[:, C:3C]  (rhs for k/v-proj, out [tok, feat])
  wq   [C, C]    bf16  Wqkv[:, :C]    (lhsT for q-proj, out [d, tok])
  wp   [C, C]    bf16  Wp             (lhsT for out-proj, out [o, tok])
  y    [C, T]    bf16  output^T (host transposes back + casts fp32)

q and gates are computed feature-major ([feat, tok]); k and v token-major
([tok, feat]) so the kv einsum can contract over tokens on the partition
axis. Gates are transposed on the DMA xbar (bf16 128x128 tiles) for the
k side. elu(z)+1 is computed as min(exp(z),1) + max(z,0), with exp and
relu on ACT and the mul/combine on DVE so neither engine gates the PE.

v2 scheduling (vs the first working version):
 - gates use k-innermost groups over (m-pair x n-set) PSUM banks so the
   first matmul only needs wg[k=0] + xt[k=0] (768KB) instead of the full
   6MB -> startup stall ~3us instead of ~19us.
 - one 8-buffer PSUM pool; all psum tiles are [128,512]f32 (1 bank).
 - elu combine on GpSimd in the k/v phase (DVE was saturated), on DVE in
   the q phase (GpSimd queue must stay clear for the collective trigger).
 - einsum for 512-token group g is emitted after kvproj chunk 4g+4 so its
   lhsT (DVE/GpSimd products) are ready when the PE reaches it.
 - attention for chunk n+1 is emitted before projection of chunk n, and
   output staging is a ping-pong buffer so the final ACT->DMA chain never
   serializes against DMA completion (this removed a ~23us tail).
"""

import numpy as np
import ml_dtypes

import concourse.bass as bass
import concourse.bacc as bacc
import concourse.tile as tile
import concourse.mybir as mybir
from concourse.bass_utils import run_bass_kernel_spmd

F32 = mybir.dt.float32
BF16 = mybir.dt.bfloat16
F8 = mybir.dt.float8e4
AF = mybir.ActivationFunctionType
ALU = mybir.AluOpType

# fp8 e4m3 scaling for the gate projection: Wg entries (~N(0, 1/1024))
# sit in e4m3's subnormal range, so scale weights x64 and x by x8 on the
# host and undo with the sigmoid's scale parameter. The sigmoid output
# error vs bf16 is negligible (verified offline: 3.0e-3 vs 2.8e-3 final).
XS = 8.0
WS = 64.0
GSCALE = 1.0 / (XS * WS)

B, N, C = 4, 4096, 1024
H, D = 16, 64
NCORES = 8
T = B * N // NCORES          # 2048 tokens per core
KC = C // 128                # 8 contraction chunks
TB = 512                     # token tile (free dim)
NT = T // TB                 # 4 token tiles
NS = T // 128                # 16 token subchunks (partition-dim tiles)
C2 = 2 * C

REPLICA_GROUPS = [[0, 1], [2, 3], [4, 5], [6, 7]]


def build_nc():
    nc = bacc.Bacc(
        "TRN2", target_bir_lowering=False, debug=False, num_devices=NCORES
    )
    xt = nc.dram_tensor("xt", [C, T], BF16, kind="ExternalInput")
    x8 = nc.dram_tensor("x8", [C, T], F8, kind="ExternalInput")
    wq = nc.dram_tensor("wq", [C, C], BF16, kind="ExternalInput")
    wkv = nc.dram_tensor("wkv", [C, C2], BF16, kind="ExternalInput")
    wg8 = nc.dram_tensor("wg8", [C, C], F8, kind="ExternalInput")
    wp = nc.dram_tensor("wp", [C, C], BF16, kind="ExternalInput")
    bgbp = nc.dram_tensor("bgbp", [128, 2 * KC], F32, kind="ExternalInput")
    e_all = nc.dram_tensor("e_all", [H, C], BF16, kind="ExternalInput")
    y = nc.dram_tensor("y", [C, T], BF16, kind="ExternalOutput")

    with tile.TileContext(nc) as tc:
        build_body(nc, tc, xt, x8, wq, wkv, wg8, wp, bgbp, e_all, y)

    nc.compile()
    return nc


def build_body(nc, tc, xt, x8, wq, wkv, wg8, wp, bgbp, e_all, y):
    from contextlib import ExitStack

    with ExitStack() as st:
        constp = st.enter_context(tc.tile_pool(name="constp", bufs=1))
        wbig = st.enter_context(tc.tile_pool(name="wbig", bufs=1))
        wsmall = st.enter_context(tc.tile_pool(name="wsmall", bufs=1))
        fp8p = st.enter_context(tc.tile_pool(name="fp8p", bufs=1))
        big1 = st.enter_context(tc.tile_pool(name="big1", bufs=1))
        gatesp = st.enter_context(tc.tile_pool(name="gatesp", bufs=1))
        qp = st.enter_context(tc.tile_pool(name="qp", bufs=1))
        workp = st.enter_context(tc.tile_pool(name="workp", bufs=2))
        elup = st.enter_context(tc.tile_pool(name="elup", bufs=2))
        psum = st.enter_context(tc.tile_pool(name="psum", bufs=8, space="PSUM"))
        dramp = st.enter_context(tc.tile_pool(name="dramp", bufs=1, space="DRAM"))

        # ---------------------------------------------- initial loads
        # emission order == sync-queue issue order; gates consume wg/xt
        # chunk pairs k-innermost, so interleave them for tight pacing.
        # bgbp goes first (the first sigmoid needs it and a late emission
        # stalls behind the big loads via DMA-semaphore recycling); wkv is
        # deferred into the gates loop so its 6MB doesn't steal wire
        # bandwidth from the wg/xt chunks the gates are waiting on.
        bgbp_sb = constp.tile([128, 2 * KC], F32, name="bgbp_sb")
        nc.sync.dma_start(bgbp_sb[:], bgbp[:])
        wg8_sb = fp8p.tile([128, KC * C], F8, name="wg8_sb", tag="wg8")
        x8_sb = fp8p.tile([128, KC * T], F8, name="x8_sb", tag="x8")
        xt_sb = big1.tile([128, KC * T], BF16, name="xt_sb", tag="big1")
        wkv_sb = wbig.tile([128, KC * C2], BF16, name="wkv_sb", tag="wbig")
        e_sb = constp.tile([H, C], BF16, name="e_sb")
        for k in range(KC):
            nc.sync.dma_start(
                wg8_sb[:, k * C : (k + 1) * C], wg8[k * 128 : (k + 1) * 128, :]
            )
            nc.sync.dma_start(
                x8_sb[:, k * T : (k + 1) * T], x8[k * 128 : (k + 1) * 128, :]
            )
        nc.sync.dma_start(e_sb[:], e_all[:])
        for k in range(KC):
            nc.sync.dma_start(
                xt_sb[:, k * T : (k + 1) * T], xt[k * 128 : (k + 1) * 128, :]
            )
        wg8r = wg8_sb.rearrange("p (k n) -> p k n", k=KC)
        x8r = x8_sb.rearrange("p (k t) -> p k t", k=KC)

        # ---------------------------------------------- phase 1: gates
        # gates[g, tok] = sigmoid(x @ Wg + bg)^T, feature-major.
        # Groups hold (m, n) PSUM banks across the k-innermost accumulation;
        # G0 spans all 4 token tiles for m-pair 0 (8 banks) to maximize
        # per-k-chunk work while the wg/xt DMA stream is still arriving.
        gates_sb = gatesp.tile([128, KC * T], BF16, name="gates_sb", tag="gates")
        gT_full = qp.tile([128, KC * T], BF16, name="gT_full", tag="gT_full")
        gT4 = gT_full.rearrange("p (m s c) -> p m s c", s=NS, c=128)

        groups = [
            (0, (0, 1, 2, 3)),
            (1, (0, 1)), (2, (0, 1)), (3, (0, 1)),
            (1, (2, 3)), (2, (2, 3)), (3, (2, 3)),
        ]
        for gi, (mp, ns) in enumerate(groups):
            ms = (2 * mp, 2 * mp + 1)
            tiles = {}
            for m in ms:
                for n in ns:
                    tiles[(m, n)] = psum.tile(
                        [128, TB], F32, name=f"gps{m}_{n}", tag="mm"
                    )
            for kp in range(KC // 2):
                for m in ms:
                    lhsT = wg8r[:, 2 * kp : 2 * kp + 2, m * 128 : (m + 1) * 128]
                    for n in ns:
                        nc.tensor.matmul(
                            tiles[(m, n)][:],
                            lhsT=lhsT,
                            rhs=x8r[:, 2 * kp : 2 * kp + 2, n * TB : (n + 1) * TB],
                            start=(kp == 0),
                            stop=(kp == KC // 2 - 1),
                            perf_mode=mybir.MatmulPerfMode.DoubleRow,
                        )
            for m in ms:
                for n in ns:
                    nc.scalar.activation(
                        gates_sb[:, m * T + n * TB : m * T + (n + 1) * TB],
                        tiles[(m, n)][:],
                        AF.Sigmoid,
                        bias=bgbp_sb[:, m : m + 1],
                        scale=GSCALE,
                    )
            # transpose the finished half-rows (or full rows for G0) on the
            # DMA xbar: gT[p, m*T + s*128 + c] = gates[m*128 + c, s*128 + p]
            span = len(ns) * TB
            base = ns[0] * TB
            for m in ms:
                nc.sync.dma_start(
                    gT_full[:, m * T + base : m * T + base + span].rearrange(
                        "p (s c) -> p s c", c=128
                    ),
                    gates_sb[:, m * T + base : m * T + base + span],
                    transpose=True,
                )
            # stream wkv in behind the gates groups (2 chunks per group)
            if gi < 4:
                for k in (2 * gi, 2 * gi + 1):
                    nc.sync.dma_start(
                        wkv_sb[:, k * C2 : (k + 1) * C2],
                        wkv[k * 128 : (k + 1) * 128, :],
                    )
            elif gi == 4:
                wq_sb = wsmall.tile([128, KC * C], BF16, name="wq_sb", tag="wsmall")
                nc.sync.dma_start(
                    wq_sb.rearrange("p (k n) -> p k n", k=KC),
                    wq.rearrange("(k p) n -> p k n", p=128),
                )

        # wp into the wsmall slot vacated by wg (waits for last gate matmul)
        wp_sb = wsmall.tile([128, KC * C], BF16, name="wp_sb", tag="wsmall")
        nc.sync.dma_start(
            wp_sb.rearrange("p (k n) -> p k n", k=KC),
            wp.rearrange("(k p) n -> p k n", p=128),
        )

        # ---------------------------------------------- phase 2: k/v + kv state
        # kv_acc block p = cols [130p, 130p+130):
        #   rows 0:64,  cols +0:65   = kv_aug head 2p   (col 64 = k_sum)
        #   rows 64:128, cols +65:130 = kv_aug head 2p+1 (col 129 = k_sum)
        kv_acc = constp.tile([128, KC * 130], F32, name="kv_acc")
        kbfs, vaugs = {}, {}

        def kvproj(s):
            kvps = [
                psum.tile([128, TB], F32, name=f"kvps{n}", tag="mm")
                for n in range(4)
            ]
            for k in range(KC):
                lhsT = xt_sb[:, k * T + s * 128 : k * T + (s + 1) * 128]
                for n in range(4):
                    nc.tensor.matmul(
                        kvps[n][:],
                        lhsT=lhsT,
                        rhs=wkv_sb[:, k * C2 + n * TB : k * C2 + (n + 1) * TB],
                        start=(k == 0),
                        stop=(k == KC - 1),
                    )
            # k = elu(k_raw * g) + 1 = min(exp(kg),1) + max(kg,0)
            k_bf = workp.tile([128, C], BF16, name="k_bf", tag="k_bf", bufs=4)
            for n in range(2):
                kg = elup.tile([128, TB], BF16, name="kg", tag="kg")
                nc.vector.tensor_mul(
                    kg.rearrange("p (m c) -> p m c", c=128),
                    kvps[n].rearrange("p (m c) -> p m c", c=128),
                    gT4[:, 4 * n : 4 * n + 4, s, :],
                )
                relu = elup.tile([128, TB], BF16, name="relu", tag="relu")
                nc.scalar.activation(relu[:], kg[:], AF.Relu)
                ex = elup.tile([128, TB], BF16, name="ex", tag="ex")
                nc.scalar.activation(ex[:], kg[:], AF.Exp)
                nc.vector.scalar_tensor_tensor(
                    k_bf[:, n * TB : (n + 1) * TB],
                    in0=ex[:],
                    scalar=1.0,
                    in1=relu[:],
                    op0=ALU.min,
                    op1=ALU.add,
                )
            # v, augmented with ones column per head (yields k_sum)
            v_aug = workp.tile(
                [128, H * 65], BF16, name="v_aug", tag="v_aug", bufs=4
            )
            v3 = v_aug.rearrange("p (h e) -> p h e", e=65)
            nc.vector.memset(v3[:, :, 64:65], 1.0)
            for n in range(2, 4):
                h0 = (n - 2) * 8
                nc.vector.tensor_copy(
                    v3[:, h0 : h0 + 8, 0:64],
                    kvps[n].rearrange("p (h e) -> p h e", e=64),
                )
            kbfs[s] = k_bf
            vaugs[s] = v_aug

        def einsum(g):
            for p in range(KC):
                eps = psum.tile([128, TB], F32, name="eps", tag="mm")
                for si in range(4):
                    s = g * 4 + si
                    nc.tensor.matmul(
                        eps[:, 0:130],
                        lhsT=kbfs[s][:, 128 * p : 128 * (p + 1)],
                        rhs=vaugs[s][:, 130 * p : 130 * (p + 1)],
                        start=(si == 0),
                        stop=(si == 3),
                    )
                if g == 0:
                    nc.vector.tensor_copy(
                        kv_acc[:, 130 * p : 130 * (p + 1)], eps[:, 0:130]
                    )
                else:
                    nc.vector.tensor_add(
                        kv_acc[:, 130 * p : 130 * (p + 1)],
                        kv_acc[:, 130 * p : 130 * (p + 1)],
                        eps[:, 0:130],
                    )

        # einsum g is emitted after kvproj 4g+4 so the PE never waits on the
        # elu chain of the group's last chunk.
        for s in range(NS):
            kvproj(s)
            if s in (5, 9, 13):
                einsum((s - 5) // 4)
        einsum(3)

        # ---------------------------------------------- kv all-reduce (pairs)
        # compact to [128, 8*65]: head 2p at [0:64, 65p:65p+65],
        # head 2p+1 at [64:128, 65p:65p+65]
        # bf16 payload: halves the wire time, and the pair-sum loses only
        # ~0.4% relative on a tensor that already tolerates bf16 downstream.
        kv_cat = constp.tile([128, KC * 65], BF16, name="kv_cat", tag="kv_cat")
        nc.vector.tensor_copy(
            kv_cat[0:64, :].rearrange("p (j e) -> p j e", e=65),
            kv_acc[0:64, :].rearrange("p (j q) -> p j q", q=130)[:, :, 0:65],
        )
        nc.vector.tensor_copy(
            kv_cat[64:128, :].rearrange("p (j e) -> p j e", e=65),
            kv_acc[64:128, :].rearrange("p (j q) -> p j q", q=130)[:, :, 65:130],
        )
        bounce_in = dramp.tile([128, KC * 65], BF16, name="bounce_in")
        bounce_out = dramp.tile([128, KC * 65], BF16, name="bounce_out")
        nc.sync.dma_start(bounce_in[:], kv_cat[:])
        nc.gpsimd.collective_compute(
            "AllReduce",
            ALU.add,
            replica_groups=REPLICA_GROUPS,
            ins=[bounce_in.opt()],
            outs=[bounce_out.opt()],
        )

        # ---------------------------------------------- phase 3: q (overlaps AR)
        # q feature-major in (m-pair x n-pair) 4-bank groups, k innermost.
        # elu combine stays on DVE here: the gpsimd queue holds the
        # collective trigger and must not be backed up behind q's work.
        q_sb = wbig.tile([128, KC * T], BF16, name="q_sb", tag="wbig")
        bds, blks, rbs = [], [], [None] * NT

        def q_group(mp, np_):
            ms = (2 * mp, 2 * mp + 1)
            ns = (2 * np_, 2 * np_ + 1)
            tiles = {}
            for m in ms:
                for n in ns:
                    tiles[(m, n)] = psum.tile(
                        [128, TB], F32, name=f"qps{m}_{n}", tag="mm"
                    )
            for k in range(KC):
                for m in ms:
                    lhsT = wq_sb[:, k * C + m * 128 : k * C + (m + 1) * 128]
                    for n in ns:
                        nc.tensor.matmul(
                            tiles[(m, n)][:],
                            lhsT=lhsT,
                            rhs=xt_sb[:, k * T + n * TB : k * T + (n + 1) * TB],
                            start=(k == 0),
                            stop=(k == KC - 1),
                        )
            for m in ms:
                for n in ns:
                    qg = elup.tile([128, TB], BF16, name="qg", tag="kg")
                    nc.vector.tensor_mul(
                        qg[:],
                        tiles[(m, n)][:],
                        gates_sb[:, m * T + n * TB : m * T + (n + 1) * TB],
                    )
                    relu = elup.tile([128, TB], BF16, name="relu2", tag="relu")
                    nc.scalar.activation(relu[:], qg[:], AF.Relu)
                    ex = elup.tile([128, TB], BF16, name="ex2", tag="ex")
                    nc.scalar.activation(ex[:], qg[:], AF.Exp)
                    nc.vector.scalar_tensor_tensor(
                        q_sb[:, m * T + n * TB : m * T + (n + 1) * TB],
                        in0=ex[:],
                        scalar=1.0,
                        in1=relu[:],
                        op0=ALU.min,
                        op1=ALU.add,
                    )

        def ar_result_prep():
            # AR result -> bf16 block-diagonal kv / k_sum tiles. Emitted in
            # the middle of the q phase so the DVE chain runs under q matmuls
            # instead of stalling the PE at the q -> attention boundary.
            kv_bf = constp.tile([128, KC * 65], BF16, name="kv_bf", tag="kv_cat")
            nc.sync.dma_start(kv_bf[:], bounce_out[:])
            for j in range(KC):
                bd = constp.tile([128, 128], BF16, name=f"bd{j}")
                nc.vector.memset(bd[:], 0.0)
                nc.vector.tensor_copy(
                    bd[0:64, 0:64], kv_bf[0:64, 65 * j : 65 * j + 64]
                )
                nc.vector.tensor_copy(
                    bd[64:128, 64:128], kv_bf[64:128, 65 * j : 65 * j + 64]
                )
                bds.append(bd)
            for j in range(KC):
                bj = constp.tile([128, H], BF16, name=f"blk{j}")
                nc.vector.memset(bj[:], 0.0)
                for par in range(2):
                    h = 2 * j + par
                    nc.vector.tensor_copy(
                        bj[par * 64 : (par + 1) * 64, h : h + 1],
                        kv_bf[par * 64 : (par + 1) * 64, 65 * j + 64 : 65 * j + 65],
                    )
                blks.append(bj)

        def norm_emit(n):
            # normalizer reciprocal for token chunk n
            nps = psum.tile([128, TB], F32, name="nps", tag="mm")
            for j in range(KC):
                nc.tensor.matmul(
                    nps[0:H, :],
                    lhsT=blks[j][:],
                    rhs=q_sb[:, j * T + n * TB : j * T + (n + 1) * TB],
                    start=(j == 0),
                    stop=(j == KC - 1),
                )
            nc.vector.tensor_scalar_add(nps[0:H, :], nps[0:H, :], 1e-8)
            nrec = elup.tile([H, TB], F32, name="nrec", tag="nrec", bufs=1)
            nc.vector.reciprocal_approx_fast(nrec[:], nps[0:H, :])
            rb = constp.tile([H, TB], BF16, name=f"rb{n}")
            nc.vector.tensor_copy(rb[:], nrec[:])
            rbs[n] = rb

        # AR-result prep (DVE-only) is emitted just before the LAST q group:
        # late enough that the in-order DVE queue reaches it after the AR has
        # completed (so it never blocks earlier q evictions), early enough
        # that bds/blks are ready when the normalizer matmuls start. All
        # normalizer matmuls come after the last q group so the PE queue
        # never parks on the AR semaphore.
        for mp in range(4):
            q_group(mp, 0)
        for mp in range(4):
            if mp == 3:
                ar_result_prep()
            q_group(mp, 1)
        for n in range(NT):
            norm_emit(n)

        # ping-pong buffers: attention output (feature-major) and staged y
        attn_pp = big1.tile([128, 2 * KC * TB], BF16, name="attn_pp", tag="big1")
        o_pp = gatesp.tile([128, 2 * KC * TB], BF16, name="o_pp", tag="gates")

        def attn_emit(n):
            # attn[e, tok] = (q @ kv) * bcast(recip)   (feature-major)
            half = (n % 2) * KC * TB
            for j in range(KC):
                pps = psum.tile([128, TB], F32, name="pps", tag="mm")
                nc.tensor.matmul(
                    pps[:],
                    lhsT=bds[j][:],
                    rhs=q_sb[:, j * T + n * TB : j * T + (n + 1) * TB],
                    start=True,
                    stop=True,
                )
                bps = psum.tile([128, TB], F32, name="bps", tag="mm")
                nc.tensor.matmul(
                    bps[:],
                    lhsT=e_sb[:, j * 128 : (j + 1) * 128],
                    rhs=rbs[n][:],
                    start=True,
                    stop=True,
                )
                # DVE can read only one PSUM operand per op: stage the
                # broadcast through SBUF (alternate ACT/DVE to balance load)
                bc_sb = elup.tile([128, TB], BF16, name="bc_sb", tag="kg")
                if j % 2 == 0:
                    nc.scalar.copy(bc_sb[:], bps[:])
                else:
                    nc.vector.tensor_copy(bc_sb[:], bps[:])
                nc.vector.tensor_mul(
                    attn_pp[:, half + j * TB : half + (j + 1) * TB],
                    pps[:],
                    bc_sb[:],
                )

        def proj_emit(n):
            # output projection for this chunk: y[o, tok] = Wp^T @ attn + bp
            half = (n % 2) * KC * TB
            for m in range(KC):
                ops_ = psum.tile([128, TB], F32, name="ops", tag="mm")
                for j in range(KC):
                    nc.tensor.matmul(
                        ops_[:],
                        lhsT=wp_sb[:, j * C + m * 128 : j * C + (m + 1) * 128],
                        rhs=attn_pp[:, half + j * TB : half + (j + 1) * TB],
                        start=(j == 0),
                        stop=(j == KC - 1),
                    )
                o_slice = o_pp[:, half + m * TB : half + (m + 1) * TB]
                nc.scalar.activation(
                    o_slice,
                    ops_[:],
                    AF.Identity,
                    bias=bgbp_sb[:, KC + m : KC + m + 1],
                    scale=1.0,
                )
                nc.sync.dma_start(
                    y[128 * m : 128 * (m + 1), n * TB : (n + 1) * TB], o_slice
                )

        attn_emit(0)
        attn_emit(1)
        proj_emit(0)
        attn_emit(2)
        proj_emit(1)
        attn_emit(3)
        proj_emit(2)
        proj_emit(3)


_NC_CACHE = {}


def get_nc():
    if "nc" not in _NC_CACHE:
        _NC_CACHE["nc"] = build_nc()
    return _NC_CACHE["nc"]


def make_in_maps(x, Wqkv, Wg, bg, Wp, bp):
    bf = ml_dtypes.bfloat16
    x = np.asarray(x, dtype=np.float32)
    Wqkv = np.asarray(Wqkv, dtype=np.float32)
    Wg = np.asarray(Wg, dtype=np.float32)
    bg = np.asarray(bg, dtype=np.float32)
    Wp = np.asarray(Wp, dtype=np.float32)
    bp = np.asarray(bp, dtype=np.float32)

    f8 = ml_dtypes.float8_e4m3fn
    wq = np.ascontiguousarray(Wqkv[:, :C]).astype(bf)
    wkv = np.ascontiguousarray(Wqkv[:, C:]).astype(bf)
    wg8 = (Wg * WS).astype(f8)
    wp = Wp.astype(bf)
    bgbp = np.concatenate(
        [bg.reshape(KC, 128).T, bp.reshape(KC, 128).T], axis=1
    )
    bgbp = np.ascontiguousarray(bgbp).astype(np.float32)
    e_all = np.zeros((H, C), dtype=bf)
    for h in range(H):
        e_all[h, h * D : (h + 1) * D] = 1.0

    xf = x.reshape(NCORES, T, C)
    in_maps = []
    for c in range(NCORES):
        xT = np.ascontiguousarray(xf[c].T)
        xtc = xT.astype(bf)
        x8c = (xT * XS).astype(f8)
        in_maps.append(
            dict(
                xt=xtc, x8=x8c, wq=wq, wkv=wkv, wg8=wg8, wp=wp,
                bgbp=bgbp, e_all=e_all,
            )
        )
    return in_maps


def kernel(x, Wqkv, Wg, bg, Wp, bp, _collect_perf=None):
    nc = get_nc()
    in_maps = make_in_maps(x, Wqkv, Wg, bg, Wp, bp)
    kwargs = {}
    if _collect_perf is not None:
        kwargs = dict(trace=True)
        if _collect_perf.get("tmpdir"):
            kwargs["tmpdir"] = _collect_perf["tmpdir"]
    res = run_bass_kernel_spmd(
        nc, in_maps, core_ids=list(range(NCORES)), **kwargs
    )
    if _collect_perf is not None:
        _collect_perf["exec_time_ns"] = res.exec_time_ns
        _collect_perf["results"] = res
    out = np.empty((NCORES, T, C), dtype=np.float32)
    for c in range(NCORES):
        out[c] = res.results[c]["y"].astype(np.float32).T
    return out.reshape(B, N, C)


# revision 25
# speedup vs baseline: 1.1507x; 1.0161x over previous
"""Gated linear attention on 8 TRN2 NeuronCores.

Sharding: data-parallel over tokens. Core c handles tokens
[c*2048, (c+1)*2048) of the flattened (B*N, C) = (16384, 1024) sequence,
i.e. batch b = c//2, sequence half = c%2. The linear-attention kv state
(and k-sum) needs a reduction over each batch's full sequence, so cores
{2b, 2b+1} all-reduce a small (128, 520) fp32 buffer (kv state + k-sum
for 16 heads) and everything else is local.

Device layouts (per core):
  xt   [C, T]    bf16  x^T shard (host pre-transposes + casts)
  wg   [C, C]    bf16  Wg   (lhsT col-blocks for gate-proj, out [g, tok])
  wkv  [C, 2C]   bf16  Wqkv[:, C:3C]  (rhs for k/v-proj, out [tok, feat])
  wq   [C, C]    bf16  Wqkv[:, :C]    (lhsT for q-proj, out [d, tok])
  wp   [C, C]    bf16  Wp             (lhsT for out-proj, out [o, tok])
  y    [C, T]    bf16  output^T (host transposes back + casts fp32)

q and gates are computed feature-major ([feat, tok]); k and v token-major
([tok, feat]) so the kv einsum can contract over tokens on the partition
axis. Gates are transposed on the DMA xbar (bf16 128x128 tiles) for the
k side. elu(z)+1 is computed as min(exp(z),1) + max(z,0), with exp and
relu on ACT and the mul/combine on DVE so neither engine gates the PE.

v2 scheduling (vs the first working version):
 - gates use k-innermost groups over (m-pair x n-set) PSUM banks so the
   first matmul only needs wg[k=0] + xt[k=0] (768KB) instead of the full
   6MB -> startup stall ~3us instead of ~19us.
 - one 8-buffer PSUM pool; all psum tiles are [128,512]f32 (1 bank).
 - elu combine on GpSimd in the k/v phase (DVE was saturated), on DVE in
   the q phase (GpSimd queue must stay clear for the collective trigger).
 - einsum for 512-token group g is emitted after kvproj chunk 4g+4 so its
   lhsT (DVE/GpSimd products) are ready when the PE reaches it.
 - attention for chunk n+1 is emitted before projection of chunk n, and
   output staging is a ping-pong buffer so the final ACT->DMA chain never
   serializes against DMA completion (this removed a ~23us tail).
"""

import numpy as np
import ml_dtypes

import concourse.bass as bass
import concourse.bacc as bacc
import concourse.tile as tile
import concourse.mybir as mybir
from concourse.bass_utils import run_bass_kernel_spmd

F32 = mybir.dt.float32
BF16 = mybir.dt.bfloat16
F8 = mybir.dt.float8e4
AF = mybir.ActivationFunctionType
ALU = mybir.AluOpType

# fp8 e4m3 scaling for the gate projection: Wg entries (~N(0, 1/1024))
# sit in e4m3's subnormal range, so scale weights x64 and x by x8 on the
# host and undo with the sigmoid's scale parameter. The sigmoid output
# error vs bf16 is negligible (verified offline: 3.0e-3 vs 2.8e-3 final).
XS = 8.0
WS = 64.0
GSCALE = 1.0 / (XS * WS)

B, N, C = 4, 4096, 1024
H, D = 16, 64
NCORES = 8
T = B * N // NCORES          # 2048 tokens per core
KC = C // 128                # 8 contraction chunks
TB = 512                     # token tile (free dim)
NT = T // TB                 # 4 token tiles
NS = T // 128                # 16 token subchunks (partition-dim tiles)
C2 = 2 * C

REPLICA_GROUPS = [[0, 1], [2, 3], [4, 5], [6, 7]]


def build_nc():
    nc = bacc.Bacc(
        "TRN2", target_bir_lowering=False, debug=False, num_devices=NCORES
    )
    xt = nc.dram_tensor("xt", [C, T], BF16, kind="ExternalInput")
    x8 = nc.dram_tensor("x8", [C, T], F8, kind="ExternalInput")
    wq = nc.dram_tensor("wq", [C, C], BF16, kind="ExternalInput")
    wkv = nc.dram_tensor("wkv", [C, C2], BF16, kind="ExternalInput")
    wg8 = nc.dram_tensor("wg8", [C, C], F8, kind="ExternalInput")
    wp = nc.dram_tensor("wp", [C, C], BF16, kind="ExternalInput")
    bgbp = nc.dram_tensor("bgbp", [128, 2 * KC], F32, kind="ExternalInput")
    e_all = nc.dram_tensor("e_all", [H, C], BF16, kind="ExternalInput")
    y = nc.dram_tensor("y", [C, T], BF16, kind="ExternalOutput")

    with tile.TileContext(nc) as tc:
        build_body(nc, tc, xt, x8, wq, wkv, wg8, wp, bgbp, e_all, y)

    nc.compile()
    return nc


def build_body(nc, tc, xt, x8, wq, wkv, wg8, wp, bgbp, e_all, y):
    from contextlib import ExitStack

    with ExitStack() as st:
        constp = st.enter_context(tc.tile_pool(name="constp", bufs=1))
        wbig = st.enter_context(tc.tile_pool(name="wbig", bufs=1))
        wsmall = st.enter_context(tc.tile_pool(name="wsmall", bufs=1))
        fp8p = st.enter_context(tc.tile_pool(name="fp8p", bufs=1))
        big1 = st.enter_context(tc.tile_pool(name="big1", bufs=1))
        gatesp = st.enter_context(tc.tile_pool(name="gatesp", bufs=1))
        qp = st.enter_context(tc.tile_pool(name="qp", bufs=1))
        workp = st.enter_context(tc.tile_pool(name="workp", bufs=2))
        elup = st.enter_context(tc.tile_pool(name="elup", bufs=2))
        psum = st.enter_context(tc.tile_pool(name="psum", bufs=8, space="PSUM"))
        dramp = st.enter_context(tc.tile_pool(name="dramp", bufs=1, space="DRAM"))

        # ---------------------------------------------- initial loads
        # emission order == sync-queue issue order; gates consume wg/xt
        # chunk pairs k-innermost, so interleave them for tight pacing.
        # bgbp goes first (the first sigmoid needs it and a late emission
        # stalls behind the big loads via DMA-semaphore recycling); wkv is
        # deferred into the gates loop so its 6MB doesn't steal wire
        # bandwidth from the wg/xt chunks the gates are waiting on.
        bgbp_sb = constp.tile([128, 2 * KC], F32, name="bgbp_sb")
        nc.sync.dma_start(bgbp_sb[:], bgbp[:])
        wg8_sb = fp8p.tile([128, KC * C], F8, name="wg8_sb", tag="wg8")
        x8_sb = fp8p.tile([128, KC * T], F8, name="x8_sb", tag="x8")
        xt_sb = big1.tile([128, KC * T], BF16, name="xt_sb", tag="big1")
        wkv_sb = wbig.tile([128, KC * C2], BF16, name="wkv_sb", tag="wbig")
        e_sb = constp.tile([H, C], BF16, name="e_sb")
        for k in range(KC):
            nc.sync.dma_start(
                wg8_sb[:, k * C : (k + 1) * C], wg8[k * 128 : (k + 1) * 128, :]
            )
            nc.sync.dma_start(
                x8_sb[:, k * T : (k + 1) * T], x8[k * 128 : (k + 1) * 128, :]
            )
        nc.sync.dma_start(e_sb[:], e_all[:])
        # xt/wkv/wq ride the Activation engine's HW-DGE queue: a second DMA
        # queue that runs in parallel with the sync one (ACT is idle until
        # the first sigmoid, and these issues carry no semaphore waits).
        for k in range(KC):
            nc.scalar.dma_start(
                xt_sb[:, k * T : (k + 1) * T], xt[k * 128 : (k + 1) * 128, :]
            )
            nc.scalar.dma_start(
                wkv_sb[:, k * C2 : (k + 1) * C2], wkv[k * 128 : (k + 1) * 128, :]
            )
        wq_sb = wsmall.tile([128, KC * C], BF16, name="wq_sb", tag="wsmall")
        nc.scalar.dma_start(
            wq_sb.rearrange("p (k n) -> p k n", k=KC),
            wq.rearrange("(k p) n -> p k n", p=128),
        )
        wg8r = wg8_sb.rearrange("p (k n) -> p k n", k=KC)
        x8r = x8_sb.rearrange("p (k t) -> p k t", k=KC)

        # ---------------------------------------------- phase 1: gates
        # gates[g, tok] = sigmoid(x @ Wg + bg)^T, feature-major.
        # Groups hold (m, n) PSUM banks across the k-innermost accumulation;
        # G0 spans all 4 token tiles for m-pair 0 (8 banks) to maximize
        # per-k-chunk work while the wg/xt DMA stream is still arriving.
        gates_sb = gatesp.tile([128, KC * T], BF16, name="gates_sb", tag="gates")
        gT_full = qp.tile([128, KC * T], BF16, name="gT_full", tag="gT_full")
        gT4 = gT_full.rearrange("p (m s c) -> p m s c", s=NS, c=128)

        groups = [
            (0, (0, 1, 2, 3)),
            (1, (0, 1)), (2, (0, 1)), (3, (0, 1)),
            (1, (2, 3)), (2, (2, 3)), (3, (2, 3)),
        ]
        for gi, (mp, ns) in enumerate(groups):
            ms = (2 * mp, 2 * mp + 1)
            tiles = {}
            for m in ms:
                for n in ns:
                    tiles[(m, n)] = psum.tile(
                        [128, TB], F32, name=f"gps{m}_{n}", tag="mm"
                    )
            for kp in range(KC // 2):
                for m in ms:
                    lhsT = wg8r[:, 2 * kp : 2 * kp + 2, m * 128 : (m + 1) * 128]
                    for n in ns:
                        nc.tensor.matmul(
                            tiles[(m, n)][:],
                            lhsT=lhsT,
                            rhs=x8r[:, 2 * kp : 2 * kp + 2, n * TB : (n + 1) * TB],
                            start=(kp == 0),
                            stop=(kp == KC // 2 - 1),
                            perf_mode=mybir.MatmulPerfMode.DoubleRow,
                        )
            for m in ms:
                for n in ns:
                    nc.scalar.activation(
                        gates_sb[:, m * T + n * TB : m * T + (n + 1) * TB],
                        tiles[(m, n)][:],
                        AF.Sigmoid,
                        bias=bgbp_sb[:, m : m + 1],
                        scale=GSCALE,
                    )
            # transpose the finished half-rows (or full rows for G0) on the
            # DMA xbar: gT[p, m*T + s*128 + c] = gates[m*128 + c, s*128 + p]
            span = len(ns) * TB
            base = ns[0] * TB
            for m in ms:
                nc.sync.dma_start(
                    gT_full[:, m * T + base : m * T + base + span].rearrange(
                        "p (s c) -> p s c", c=128
                    ),
                    gates_sb[:, m * T + base : m * T + base + span],
                    transpose=True,
                )
        # wp reuses x8's slot (same 16KB, and x8 dies with the last gate
        # matmul) so its DMA issues right at gates-end without blocking the
        # sync queue on a long-lived buffer.
        wp_sb = fp8p.tile([128, KC * C], BF16, name="wp_sb", tag="x8")
        nc.sync.dma_start(
            wp_sb.rearrange("p (k n) -> p k n", k=KC),
            wp.rearrange("(k p) n -> p k n", p=128),
        )

        # ---------------------------------------------- phase 2: k/v + kv state
        # kv_acc block p = cols [130p, 130p+130):
        #   rows 0:64,  cols +0:65   = kv_aug head 2p   (col 64 = k_sum)
        #   rows 64:128, cols +65:130 = kv_aug head 2p+1 (col 129 = k_sum)
        kv_acc = constp.tile([128, KC * 130], F32, name="kv_acc")
        kbfs, vaugs = {}, {}

        def kvproj(s):
            kvps = [
                psum.tile([128, TB], F32, name=f"kvps{n}", tag="mm")
                for n in range(4)
            ]
            for k in range(KC):
                lhsT = xt_sb[:, k * T + s * 128 : k * T + (s + 1) * 128]
                for n in range(4):
                    nc.tensor.matmul(
                        kvps[n][:],
                        lhsT=lhsT,
                        rhs=wkv_sb[:, k * C2 + n * TB : k * C2 + (n + 1) * TB],
                        start=(k == 0),
                        stop=(k == KC - 1),
                    )
            # k = elu(k_raw * g) + 1 = min(exp(kg),1) + max(kg,0)
            k_bf = workp.tile([128, C], BF16, name="k_bf", tag="k_bf", bufs=4)
            for n in range(2):
                kg = elup.tile([128, TB], BF16, name="kg", tag="kg")
                nc.vector.tensor_mul(
                    kg.rearrange("p (m c) -> p m c", c=128),
                    kvps[n].rearrange("p (m c) -> p m c", c=128),
                    gT4[:, 4 * n : 4 * n + 4, s, :],
                )
                relu = elup.tile([128, TB], BF16, name="relu", tag="relu")
                nc.scalar.activation(relu[:], kg[:], AF.Relu)
                ex = elup.tile([128, TB], BF16, name="ex", tag="ex")
                nc.scalar.activation(ex[:], kg[:], AF.Exp)
                nc.vector.scalar_tensor_tensor(
                    k_bf[:, n * TB : (n + 1) * TB],
                    in0=ex[:],
                    scalar=1.0,
                    in1=relu[:],
                    op0=ALU.min,
                    op1=ALU.add,
                )
            # v, augmented with ones column per head (yields k_sum)
            v_aug = workp.tile(
                [128, H * 65], BF16, name="v_aug", tag="v_aug", bufs=4
            )
            v3 = v_aug.rearrange("p (h e) -> p h e", e=65)
            nc.vector.memset(v3[:, :, 64:65], 1.0)
            for n in range(2, 4):
                h0 = (n - 2) * 8
                nc.vector.tensor_copy(
                    v3[:, h0 : h0 + 8, 0:64],
                    kvps[n].rearrange("p (h e) -> p h e", e=64),
                )
            kbfs[s] = k_bf
            vaugs[s] = v_aug

        def einsum(g):
            for p in range(KC):
                eps = psum.tile([128, TB], F32, name="eps", tag="mm")
                for si in range(4):
                    s = g * 4 + si
                    nc.tensor.matmul(
                        eps[:, 0:130],
                        lhsT=kbfs[s][:, 128 * p : 128 * (p + 1)],
                        rhs=vaugs[s][:, 130 * p : 130 * (p + 1)],
                        start=(si == 0),
                        stop=(si == 3),
                    )
                if g == 0:
                    nc.vector.tensor_copy(
                        kv_acc[:, 130 * p : 130 * (p + 1)], eps[:, 0:130]
                    )
                else:
                    nc.vector.tensor_add(
                        kv_acc[:, 130 * p : 130 * (p + 1)],
                        kv_acc[:, 130 * p : 130 * (p + 1)],
                        eps[:, 0:130],
                    )

        # einsum g is emitted after kvproj 4g+4 so the PE never waits on the
        # elu chain of the group's last chunk.
        for s in range(NS):
            kvproj(s)
            if s in (5, 9, 13):
                einsum((s - 5) // 4)
        einsum(3)

        # ---------------------------------------------- kv all-reduce (pairs)
        # compact to [128, 8*65]: head 2p at [0:64, 65p:65p+65],
        # head 2p+1 at [64:128, 65p:65p+65]
        # bf16 payload: halves the wire time, and the pair-sum loses only
        # ~0.4% relative on a tensor that already tolerates bf16 downstream.
        kv_cat = constp.tile([128, KC * 65], BF16, name="kv_cat", tag="kv_cat")
        nc.vector.tensor_copy(
            kv_cat[0:64, :].rearrange("p (j e) -> p j e", e=65),
            kv_acc[0:64, :].rearrange("p (j q) -> p j q", q=130)[:, :, 0:65],
        )
        nc.vector.tensor_copy(
            kv_cat[64:128, :].rearrange("p (j e) -> p j e", e=65),
            kv_acc[64:128, :].rearrange("p (j q) -> p j q", q=130)[:, :, 65:130],
        )
        bounce_in = dramp.tile([128, KC * 65], BF16, name="bounce_in")
        bounce_out = dramp.tile([128, KC * 65], BF16, name="bounce_out")
        nc.sync.dma_start(bounce_in[:], kv_cat[:])
        nc.gpsimd.collective_compute(
            "AllReduce",
            ALU.add,
            replica_groups=REPLICA_GROUPS,
            ins=[bounce_in.opt()],
            outs=[bounce_out.opt()],
        )

        # ---------------------------------------------- phase 3: q (overlaps AR)
        # q feature-major in (m-pair x n-pair) 4-bank groups, k innermost.
        # elu combine stays on DVE here: the gpsimd queue holds the
        # collective trigger and must not be backed up behind q's work.
        q_sb = wbig.tile([128, KC * T], BF16, name="q_sb", tag="wbig")
        bds, blks, rbs = [], [], [None] * NT

        def q_group(mp, np_):
            ms = (2 * mp, 2 * mp + 1)
            ns = (2 * np_, 2 * np_ + 1)
            tiles = {}
            for m in ms:
                for n in ns:
                    tiles[(m, n)] = psum.tile(
                        [128, TB], F32, name=f"qps{m}_{n}", tag="mm"
                    )
            for k in range(KC):
                for m in ms:
                    lhsT = wq_sb[:, k * C + m * 128 : k * C + (m + 1) * 128]
                    for n in ns:
                        nc.tensor.matmul(
                            tiles[(m, n)][:],
                            lhsT=lhsT,
                            rhs=xt_sb[:, k * T + n * TB : k * T + (n + 1) * TB],
                            start=(k == 0),
                            stop=(k == KC - 1),
                        )
            for m in ms:
                for n in ns:
                    qg = elup.tile([128, TB], BF16, name="qg", tag="kg")
                    nc.vector.tensor_mul(
                        qg[:],
                        tiles[(m, n)][:],
                        gates_sb[:, m * T + n * TB : m * T + (n + 1) * TB],
                    )
                    relu = elup.tile([128, TB], BF16, name="relu2", tag="relu")
                    nc.scalar.activation(relu[:], qg[:], AF.Relu)
                    ex = elup.tile([128, TB], BF16, name="ex2", tag="ex")
                    nc.scalar.activation(ex[:], qg[:], AF.Exp)
                    nc.vector.scalar_tensor_tensor(
                        q_sb[:, m * T + n * TB : m * T + (n + 1) * TB],
                        in0=ex[:],
                        scalar=1.0,
                        in1=relu[:],
                        op0=ALU.min,
                        op1=ALU.add,
                    )

        def ar_result_prep():
            # AR result -> bf16 block-diagonal kv / k_sum tiles. Emitted in
            # the middle of the q phase so the DVE chain runs under q matmuls
            # instead of stalling the PE at the q -> attention boundary.
            kv_bf = constp.tile([128, KC * 65], BF16, name="kv_bf", tag="kv_cat")
            nc.sync.dma_start(kv_bf[:], bounce_out[:])
            for j in range(KC):
                bd = constp.tile([128, 128], BF16, name=f"bd{j}")
                nc.vector.memset(bd[:], 0.0)
                nc.vector.tensor_copy(
                    bd[0:64, 0:64], kv_bf[0:64, 65 * j : 65 * j + 64]
                )
                nc.vector.tensor_copy(
                    bd[64:128, 64:128], kv_bf[64:128, 65 * j : 65 * j + 64]
                )
                bds.append(bd)
            for j in range(KC):
                bj = constp.tile([128, H], BF16, name=f"blk{j}")
                nc.vector.memset(bj[:], 0.0)
                for par in range(2):
                    h = 2 * j + par
                    nc.vector.tensor_copy(
                        bj[par * 64 : (par + 1) * 64, h : h + 1],
                        kv_bf[par * 64 : (par + 1) * 64, 65 * j + 64 : 65 * j + 65],
                    )
                blks.append(bj)

        def norm_emit(n):
            # normalizer reciprocal for token chunk n
            nps = psum.tile([128, TB], F32, name="nps", tag="mm")
            for j in range(KC):
                nc.tensor.matmul(
                    nps[0:H, :],
                    lhsT=blks[j][:],
                    rhs=q_sb[:, j * T + n * TB : j * T + (n + 1) * TB],
                    start=(j == 0),
                    stop=(j == KC - 1),
                )
            nc.vector.tensor_scalar_add(nps[0:H, :], nps[0:H, :], 1e-8)
            nrec = elup.tile([H, TB], F32, name="nrec", tag="nrec", bufs=1)
            nc.vector.reciprocal_approx_fast(nrec[:], nps[0:H, :])
            rb = constp.tile([H, TB], BF16, name=f"rb{n}")
            nc.vector.tensor_copy(rb[:], nrec[:])
            rbs[n] = rb

        # AR-result prep (DVE-only) is emitted just before the LAST q group:
        # late enough that the in-order DVE queue reaches it after the AR has
        # completed (so it never blocks earlier q evictions), early enough
        # that bds/blks are ready when the normalizer matmuls start. All
        # normalizer matmuls come after the last q group so the PE queue
        # never parks on the AR semaphore.
        for mp in range(4):
            q_group(mp, 0)
        for mp in range(4):
            if mp == 3:
                ar_result_prep()
            q_group(mp, 1)
        for n in range(NT):
            norm_emit(n)

        # ping-pong buffers: attention output (feature-major) and staged y
        attn_pp = big1.tile([128, 2 * KC * TB], BF16, name="attn_pp", tag="big1")
        o_pp = gatesp.tile([128, 2 * KC * TB], BF16, name="o_pp", tag="gates")

        def attn_emit(n):
            # attn[e, tok] = (q @ kv) * bcast(recip)   (feature-major)
            half = (n % 2) * KC * TB
            for j in range(KC):
                pps = psum.tile([128, TB], F32, name="pps", tag="mm")
                nc.tensor.matmul(
                    pps[:],
                    lhsT=bds[j][:],
                    rhs=q_sb[:, j * T + n * TB : j * T + (n + 1) * TB],
                    start=True,
                    stop=True,
                )
                bps = psum.tile([128, TB], F32, name="bps", tag="mm")
                nc.tensor.matmul(
                    bps[:],
                    lhsT=e_sb[:, j * 128 : (j + 1) * 128],
                    rhs=rbs[n][:],
                    start=True,
                    stop=True,
                )
                # DVE can read only one PSUM operand per op: stage the
                # broadcast through SBUF (alternate ACT/DVE to balance load)
                bc_sb = elup.tile([128, TB], BF16, name="bc_sb", tag="kg")
                if j % 2 == 0:
                    nc.scalar.copy(bc_sb[:], bps[:])
                else:
                    nc.vector.tensor_copy(bc_sb[:], bps[:])
                nc.vector.tensor_mul(
                    attn_pp[:, half + j * TB : half + (j + 1) * TB],
                    pps[:],
                    bc_sb[:],
                )

        def proj_emit(n):
            # output projection for this chunk: y[o, tok] = Wp^T @ attn + bp
            half = (n % 2) * KC * TB
            for m in range(KC):
                ops_ = psum.tile([128, TB], F32, name="ops", tag="mm")
                for j in range(KC):
                    nc.tensor.matmul(
                        ops_[:],
                        lhsT=wp_sb[:, j * C + m * 128 : j * C + (m + 1) * 128],
                        rhs=attn_pp[:, half + j * TB : half + (j + 1) * TB],
                        start=(j == 0),
                        stop=(j == KC - 1),
                    )
                o_slice = o_pp[:, half + m * TB : half + (m + 1) * TB]
                nc.scalar.activation(
                    o_slice,
                    ops_[:],
                    AF.Identity,
                    bias=bgbp_sb[:, KC + m : KC + m + 1],
                    scale=1.0,
                )
                nc.sync.dma_start(
                    y[128 * m : 128 * (m + 1), n * TB : (n + 1) * TB], o_slice
                )

        attn_emit(0)
        attn_emit(1)
        proj_emit(0)
        attn_emit(2)
        proj_emit(1)
        attn_emit(3)
        proj_emit(2)
        proj_emit(3)


_NC_CACHE = {}


def get_nc():
    if "nc" not in _NC_CACHE:
        _NC_CACHE["nc"] = build_nc()
    return _NC_CACHE["nc"]


def make_in_maps(x, Wqkv, Wg, bg, Wp, bp):
    bf = ml_dtypes.bfloat16
    x = np.asarray(x, dtype=np.float32)
    Wqkv = np.asarray(Wqkv, dtype=np.float32)
    Wg = np.asarray(Wg, dtype=np.float32)
    bg = np.asarray(bg, dtype=np.float32)
    Wp = np.asarray(Wp, dtype=np.float32)
    bp = np.asarray(bp, dtype=np.float32)

    f8 = ml_dtypes.float8_e4m3fn
    wq = np.ascontiguousarray(Wqkv[:, :C]).astype(bf)
    wkv = np.ascontiguousarray(Wqkv[:, C:]).astype(bf)
    wg8 = (Wg * WS).astype(f8)
    wp = Wp.astype(bf)
    bgbp = np.concatenate(
        [bg.reshape(KC, 128).T, bp.reshape(KC, 128).T], axis=1
    )
    bgbp = np.ascontiguousarray(bgbp).astype(np.float32)
    e_all = np.zeros((H, C), dtype=bf)
    for h in range(H):
        e_all[h, h * D : (h + 1) * D] = 1.0

    xf = x.reshape(NCORES, T, C)
    in_maps = []
    for c in range(NCORES):
        xT = np.ascontiguousarray(xf[c].T)
        xtc = xT.astype(bf)
        x8c = (xT * XS).astype(f8)
        in_maps.append(
            dict(
                xt=xtc, x8=x8c, wq=wq, wkv=wkv, wg8=wg8, wp=wp,
                bgbp=bgbp, e_all=e_all,
            )
        )
    return in_maps


def kernel(x, Wqkv, Wg, bg, Wp, bp, _collect_perf=None):
    nc = get_nc()
    in_maps = make_in_maps(x, Wqkv, Wg, bg, Wp, bp)
    kwargs = {}
    if _collect_perf is not None:
        kwargs = dict(trace=True)
        if _collect_perf.get("tmpdir"):
            kwargs["tmpdir"] = _collect_perf["tmpdir"]
    res = run_bass_kernel_spmd(
        nc, in_maps, core_ids=list(range(NCORES)), **kwargs
    )
    if _collect_perf is not None:
        _collect_perf["exec_time_ns"] = res.exec_time_ns
        _collect_perf["results"] = res
    out = np.empty((NCORES, T, C), dtype=np.float32)
    for c in range(NCORES):
        out[c] = res.results[c]["y"].astype(np.float32).T
    return out.reshape(B, N, C)
